# revision 1
# baseline (speedup 1.0000x reference)
"""PointTransformerLayer Bass kernel for TRN2.

Design (per core, points sharded across 8 cores):
  - Neighbor data comes from a packed DRAM table: one 512B row per point =
    [feats fp16 (128) | a fp16 (3) | zero pad], where a = pos @ Afold with the
    first linear_p layer + BN folded in (host-side parameter folding).
  - dma_gather int16 indices only reach 32768 rows, so the table is split
    lo/hi with a zero row at index 0 of each; out-of-range indices map to the
    zero row and the two gathered tiles are merged with one DVE add.
  - Transpose-mode gather lands channels on partitions: G[c, (pt,k)] -- the
    layout every downstream matmul wants.
  - All BN layers are affine in eval mode and folded into weights/per-channel
    biases (applied via ACT per-partition scale/bias).
  - Softmax logits are computed with Ww2 column-tiled 8x so exp() output IS
    the [128, n] broadcast weight tile; sum/normalize happen post-reduction.
  - Residual + leaky_relu in two small DVE ops; output written transposed and
    fixed up on the host.
"""

import sys

sys.path.insert(0, "/opt/trn_rl_repo")
sys.path.insert(0, "/root/.axon_site/_ro/trn_rl_repo")

import numpy as np

import concourse.bass as bass
import concourse.tile as tile
from concourse import library_config, mybir

F16 = mybir.dt.float16
F32 = mybir.dt.float32
I16 = mybir.dt.int16

K = 16
C = 128
S = 8
CS = C // S  # 16
EPS = 1e-5
EXP_SHIFT = float(np.log(256.0))  # subtracted from logits before exp
PT_TILE = 128          # points per tile
NPAIR = PT_TILE * K    # 2048 gather columns per tile
CHUNK = 1024           # psum column chunk
QHI = 0                # hw: only swdge queue 0 is serviced
MM = 512               # moving-operand columns per matmul


# ----------------------------------------------------------------- host math
def fold_params(p):
    """Fold BN params / biases. Tiny O(C^2) parameter-only preprocessing."""
    f32 = np.float32
    s_p = (p["p_gamma"] / np.sqrt(p["p_var"] + EPS)).astype(f32)
    Afold = (p["Wp1"] * s_p[None, :]).astype(f32)
    cfold = ((p["bp1"] - p["p_mean"]) * s_p + p["p_beta"]).astype(f32)

    s_w = (p["w_gamma"] / np.sqrt(p["w_var"] + EPS)).astype(f32)
    ball = (p["bk"] - p["bq"] + p["bp2"]).astype(f32)
    b_w = ((ball - p["w_mean"]) * s_w + p["w_beta"]).astype(f32)

    s1 = (p["w1_gamma"] / np.sqrt(p["w1_var"] + EPS)).astype(f32)
    ww1s = (p["Ww1"] * s1[None, :]).astype(np.float16)
    b1f = ((p["bw1"] - p["w1_mean"]) * s1 + p["w1_beta"]).astype(f32)

    ww2r = np.tile(p["Ww2"], (1, S)).astype(np.float16)          # [16, 128]
    be_bias = (np.tile(p["bw2"], S) - EXP_SHIFT).astype(f32)      # [128]
    bvp = (p["bv"] + p["bp2"]).astype(f32)                        # [128]

    return dict(
        wk=p["Wk"].astype(np.float16),
        wv=p["Wv"].astype(np.float16),
        wqn=(-p["Wq"]).astype(np.float16),
        wp2=p["Wp2"].astype(np.float16),  # [3, 128]
        ww1s=ww1s, ww2r=ww2r,
        s_w=s_w, b_w=b_w, b1f=b1f, be_bias=be_bias, bvp=bvp,
        Afold=Afold, cfold=cfold,
    )


def prep_inputs(xyz, feats, nei_ind, params, n_cores, thresh):
    """Build per-core in_maps. Host work is slicing / transposes / dtype
    conversion plus the tiny parameter folds above."""
    f = fold_params(params)
    n_real = feats.shape[1]
    per_core_raw = -(-n_real // n_cores)
    per_core = -(-per_core_raw // PT_TILE) * PT_TILE
    npad = per_core * n_cores
    n_tiles = per_core // PT_TILE

    feats0 = np.zeros((npad, C), np.float32)
    feats0[:n_real] = feats[0]
    pos0 = np.zeros((npad, 3), np.float32)
    pos0[:n_real] = xyz[0]
    ni = np.zeros((npad, K), np.int64)
    ni[:n_real] = nei_ind[0]

    a = (pos0 @ f["Afold"]).astype(np.float32)            # [npad, 3]
    actrC = (a - f["cfold"][None, :]).astype(np.float32)  # center role

    # packed table rows: [feats f16 (128) | a f16 (3) | pad] = 256 f16 = 512B
    ent = np.zeros((npad, 256), np.float16)
    ent[:, :C] = feats0.astype(np.float16)
    ent[:, C:C + 3] = a.astype(np.float16)

    lo_rows = thresh + 1
    hi_rows = npad - thresh + 1
    table_lo = np.zeros((lo_rows, 256), np.float16)
    table_lo[1:] = ent[:thresh]
    table_hi = np.zeros((hi_rows, 256), np.float16)
    table_hi[1:] = ent[thresh:]

    lo_all = np.where(ni < thresh, ni + 1, 0).astype(np.int16)        # [npad, K]
    hi_all = np.where(ni >= thresh, ni - thresh + 1, 0).astype(np.int16)

    featsT = np.ascontiguousarray(feats0.T)               # [C, npad] f32
    actrT = np.ascontiguousarray(actrC.T)                 # [3, npad] f32

    def wrap_idx(arr_core):
        # arr_core: [per_core, K] -> [128, n_tiles*128] int16 in the
        # (s p)-wrapped layout dma_gather expects, replicated to 8 groups.
        out = np.zeros((128, n_tiles * 128), np.int16)
        for t in range(n_tiles):
            flat = arr_core[t * PT_TILE:(t + 1) * PT_TILE].reshape(-1)  # 2048
            w16 = flat.reshape(128, 16).T                                # [16,128]
            out[:, t * 128:(t + 1) * 128] = np.tile(w16, (8, 1))
        return out

    in_maps = []
    for c in range(n_cores):
        sl = slice(c * per_core, (c + 1) * per_core)
        actrE = np.repeat(actrT[:, sl].astype(np.float16), K, axis=1)  # [3, per_core*K]
        in_maps.append({
            "table_lo": table_lo, "table_hi": table_hi,
            "idx_lo": wrap_idx(lo_all[sl]), "idx_hi": wrap_idx(hi_all[sl]),
            "featsT": np.ascontiguousarray(featsT[:, sl]),
            "actrE": np.ascontiguousarray(actrE),
            "wk": f["wk"], "wv": f["wv"], "wqn": f["wqn"], "wp2": f["wp2"],
            "ww1s": f["ww1s"], "ww2r": f["ww2r"],
            "s_w": f["s_w"].reshape(C, 1), "b_w": f["b_w"].reshape(C, 1),
            "b1f": f["b1f"].reshape(CS, 1),
            "be_bias": f["be_bias"].reshape(C, 1),
            "bvp": f["bvp"].reshape(C, 1),
        })
    meta = dict(n_tiles=n_tiles, per_core=per_core, npad=npad,
                lo_rows=lo_rows, hi_rows=hi_rows, n_real=n_real)
    return in_maps, meta


# ------------------------------------------------------------- walrus compat
def split_excess_waits(nc, max_waits=1):
    """This walrus build allows only 1 sync wait on CTRL instructions
    (Drain/NoOp) and a few on compute instructions. Move excess waits onto
    preceding single-wait NoOps."""
    n_split = 0
    for fn in nc.m.functions:
        for blk in fn.blocks:
            new_insts = []
            for inst in blk.instructions:
                si = inst.sync_info
                lim = (1 if isinstance(inst, (mybir.InstDrain, mybir.InstNoOp,
                                              mybir.InstEventSemaphore))
                       else max_waits)
                if si is not None and si.on_wait and len(si.on_wait) > lim:
                    waits = list(si.on_wait)
                    extra, keep = waits[:-lim], waits[-lim:]
                    ci = 0
                    while extra:
                        chunk, extra = extra[:1], extra[1:]
                        new_insts.append(mybir.InstNoOp(
                            name=f"{inst.name}-waitsplit{ci}",
                            engine=inst.engine,
                            bass_nofuse=True,
                            sync_info=mybir.SyncInfo(on_wait=chunk, on_update=[]),
                        ))
                        ci += 1
                    si.on_wait = keep
                    n_split += 1
                new_insts.append(inst)
            blk.instructions = new_insts
    return n_split


# ----------------------------------------------------------------- the kernel
def build_nc(meta, enable_asserts=False, pe_bcast=True, split_waits=True):
    n_tiles = meta["n_tiles"]
    per_core = meta["per_core"]
    nc = bass.Bass("TRN2", target_bir_lowering=False, debug=False,
                   enable_asserts=enable_asserts, num_swdge_queues=1)

    dt_ = nc.dram_tensor
    t_lo = dt_("table_lo", [meta["lo_rows"], 256], F16, kind="ExternalInput").ap()
    t_hi = dt_("table_hi", [meta["hi_rows"], 256], F16, kind="ExternalInput").ap()
    idx_lo = dt_("idx_lo", [128, n_tiles * 128], I16, kind="ExternalInput").ap()
    idx_hi = dt_("idx_hi", [128, n_tiles * 128], I16, kind="ExternalInput").ap()
    featsT = dt_("featsT", [C, per_core], F32, kind="ExternalInput").ap()
    actrE = dt_("actrE", [3, per_core * K], F16, kind="ExternalInput").ap()
    wk_d = dt_("wk", [C, C], F16, kind="ExternalInput").ap()
    wv_d = dt_("wv", [C, C], F16, kind="ExternalInput").ap()
    wqn_d = dt_("wqn", [C, C], F16, kind="ExternalInput").ap()
    wp2_d = dt_("wp2", [3, C], F16, kind="ExternalInput").ap()
    ww1s_d = dt_("ww1s", [C, CS], F16, kind="ExternalInput").ap()
    ww2r_d = dt_("ww2r", [CS, C], F16, kind="ExternalInput").ap()
    s_w_d = dt_("s_w", [C, 1], F32, kind="ExternalInput").ap()
    b_w_d = dt_("b_w", [C, 1], F32, kind="ExternalInput").ap()
    b1f_d = dt_("b1f", [CS, 1], F32, kind="ExternalInput").ap()
    be_d = dt_("be_bias", [C, 1], F32, kind="ExternalInput").ap()
    bvp_d = dt_("bvp", [C, 1], F32, kind="ExternalInput").ap()
    outT = dt_("outT", [C, per_core], F32, kind="ExternalOutput").ap()

    Relu = mybir.ActivationFunctionType.Relu
    Exp = mybir.ActivationFunctionType.Exp
    ADD = mybir.AluOpType.add
    MULT = mybir.AluOpType.mult
    SUB = mybir.AluOpType.subtract
    MAX = mybir.AluOpType.max

    nc.gpsimd.load_library(library_config.mlp)
    nidx_reg = nc.gpsimd.alloc_register("nidx")
    nc.gpsimd.reg_mov(nidx_reg, NPAIR)

    with tile.TileContext(nc) as tc:
        with (
            tc.tile_pool(name="const", bufs=1) as cpool,
            tc.tile_pool(name="gath", bufs=2) as gpool,
            tc.tile_pool(name="gm", bufs=2) as gmpool,
            tc.tile_pool(name="xs", bufs=2) as xpool,
            tc.tile_pool(name="mid", bufs=2) as mpool,
            tc.tile_pool(name="tail", bufs=2) as tpool,
            tc.tile_pool(name="psA", bufs=4, space="PSUM") as psA,
        ):
            # ---- constants into SBUF once
            def cload(ap_dram, shape, dtype, tag):
                t = cpool.tile(shape, dtype, tag=tag)
                nc.sync.dma_start(t[:], ap_dram)
                return t

            wk = cload(wk_d, [C, C], F16, "wk")
            wv = cload(wv_d, [C, C], F16, "wv")
            wqn = cload(wqn_d, [C, C], F16, "wqn")
            wp2 = cload(wp2_d, [3, C], F16, "wp2")
            ww1s = cload(ww1s_d, [C, CS], F16, "ww1s")
            ww2r = cload(ww2r_d, [CS, C], F16, "ww2r")
            s_w = cload(s_w_d, [C, 1], F32, "s_w")
            b_w = cload(b_w_d, [C, 1], F32, "b_w")
            b1f = cload(b1f_d, [CS, 1], F32, "b1f")
            be_b = cload(be_d, [C, 1], F32, "be_b")
            bvp = cload(bvp_d, [C, 1], F32, "bvp")
            ixlo = cload(idx_lo, [128, n_tiles * 128], I16, "ixlo")
            ixhi = cload(idx_hi, [128, n_tiles * 128], I16, "ixhi")

            # whole-core featsT resident in SBUF (one DMA, 20KB/partition)
            ftw = cpool.tile([C, per_core], F32, tag="ftw")
            nc.sync.dma_start(ftw[:], featsT)

            ACHUNK = 2  # tiles per actrE load
            act_ch = None

            state = {}

            def s0_gather(t):
                cols = bass.ts(t, 128)
                glo = gpool.tile([128, 2, NPAIR], F16, tag="glo")
                nc.gpsimd.dma_gather(glo[:], t_lo, ixlo[:, cols], NPAIR, nidx_reg,
                                     256, transpose=True, queue_num=0,
                                     single_packet=False)
                ghi = gpool.tile([128, 2, NPAIR], F16, tag="ghi")
                nc.gpsimd.dma_gather(ghi[:], t_hi, ixhi[:, cols], NPAIR, nidx_reg,
                                     256, transpose=True, queue_num=QHI,
                                     single_packet=False)
                if t % ACHUNK == 0:
                    nch = min(ACHUNK, n_tiles - t)
                    act_ch = xpool.tile([3, ACHUNK * NPAIR], F16, tag="act")
                    nc.sync.dma_start(act_ch[:, :nch * NPAIR],
                                      actrE[:, t * NPAIR:(t + nch) * NPAIR])
                    state["act_ch"] = act_ch
                state[("g", t)] = (glo, ghi, state["act_ch"])

            def s1_front(t):
                glo, ghi, act_ch = state.pop(("g", t))
                act = act_ch[:, (t % ACHUNK) * NPAIR:(t % ACHUNK + 1) * NPAIR]
                gm = gmpool.tile([128, 2, NPAIR], F16, tag="gm")
                nc.vector.tensor_tensor(gm[:], glo[:], ghi[:], ADD)
                xT16 = xpool.tile([C, 128], F16, tag="xT16")
                nc.vector.tensor_copy(xT16[:], ftw[:, bass.ts(t, 128)])
                u = mpool.tile([3, NPAIR], F16, tag="u")
                nc.vector.scalar_tensor_tensor(u[:], gm[0:3, 1, :], 0.0, act,
                                               ADD, SUB)
                ru = mpool.tile([3, NPAIR], F16, tag="ru")
                nc.vector.tensor_scalar(ru[:], u[:], 0.0, None, MAX)
                state[("f", t)] = (gm, xT16, ru)

            def s2_chunks(t):
                gm, xT16, ru = state.pop(("f", t))
                gf = gm[:, 0, :]
                r_t = mpool.tile([C, NPAIR], F16, tag="r")
                h2_t = mpool.tile([CS, NPAIR], F16, tag="h2")
                e_t = mpool.tile([C, NPAIR], F16, tag="e")
                t2_t = mpool.tile([C, NPAIR], F16, tag="t2")
                v16_t = mpool.tile([C, NPAIR], F16, tag="v16")
                nmm = CHUNK // MM

                def gf_sl(lo, n):
                    return gf[:, lo:lo + n]

                def ru_sl(lo, n):
                    return ru[:, lo:lo + n]

                def q_sl(lo, n):
                    p0 = lo // K
                    return (xT16[:, p0:p0 + n // K]
                            .unsqueeze(2).broadcast_to([C, n // K, K]))

                for ch in range(NPAIR // CHUNK):
                    csl = bass.ts(ch, CHUNK)
                    wps = psA.tile([C, CHUNK], F32, tag="big")
                    specs_w = [(wk[:], gf_sl), (wqn[:], q_sl), (wp2[:], ru_sl)]
                    for wi, (lhsT, rfn) in enumerate(specs_w):
                        for hf in range(nmm):
                            lo = ch * CHUNK + hf * MM
                            nc.tensor.matmul(
                                wps[:, hf * MM:(hf + 1) * MM], lhsT, rfn(lo, MM),
                                start=(wi == 0), stop=(wi == len(specs_w) - 1))
                    nc.scalar.activation(r_t[:, csl], wps[:], Relu,
                                         bias=b_w[:], scale=s_w[:])
                    hps_full = psA.tile([C, CHUNK], F32, tag="big")
                    hps = hps_full[0:CS, :]
                    for hf in range(nmm):
                        lo = ch * CHUNK + hf * MM
                        nc.tensor.matmul(hps[:, hf * MM:(hf + 1) * MM], ww1s[:],
                                         r_t[:, lo:lo + MM],
                                         start=True, stop=True)
                    nc.scalar.activation(h2_t[:, csl], hps[:], Relu, bias=b1f[:])
                    lps = psA.tile([C, CHUNK], F32, tag="big")
                    for hf in range(nmm):
                        lo = ch * CHUNK + hf * MM
                        nc.tensor.matmul(lps[:, hf * MM:(hf + 1) * MM], ww2r[:],
                                         h2_t[:, lo:lo + MM],
                                         start=True, stop=True)
                    nc.scalar.activation(e_t[:, csl], lps[:], Exp, bias=be_b[:])
                    vps = psA.tile([C, CHUNK], F32, tag="big")
                    specs_v = [(wv[:], gf_sl), (wp2[:], ru_sl)]
                    for wi, (lhsT, rfn) in enumerate(specs_v):
                        for hf in range(nmm):
                            lo = ch * CHUNK + hf * MM
                            nc.tensor.matmul(
                                vps[:, hf * MM:(hf + 1) * MM], lhsT, rfn(lo, MM),
                                start=(wi == 0), stop=(wi == len(specs_v) - 1))
                    nc.scalar.copy(v16_t[:, csl], vps[:])
                    nc.vector.tensor_tensor(t2_t[:, csl], e_t[:, csl],
                                            v16_t[:, csl], MULT)
                state[("c", t)] = (e_t, t2_t)

            def s3_tail(t):
                e_t, t2_t = state.pop(("c", t))

                def ktree(src_t, out32, tagp):
                    cur = src_t[:].rearrange("p (a b) -> p a b", b=K)
                    kk = K
                    while kk > 2:
                        nx = tpool.tile([C, 128 * kk // 2], F16, tag=f"{tagp}{kk}")
                        nxv = nx[:].rearrange("p (a b) -> p a b", b=kk // 2)
                        nc.vector.tensor_tensor(
                            nxv, cur[:, :, 0:kk // 2], cur[:, :, kk // 2:kk], ADD)
                        cur, kk = nxv, kk // 2
                    nc.vector.tensor_tensor(out32[:], cur[:, :, 0], cur[:, :, 1], ADD)

                S_t = tpool.tile([C, 128], F32, tag="S")
                ktree(e_t, S_t, "se")
                aggU = tpool.tile([C, 128], F32, tag="aggU")
                ktree(t2_t, aggU, "sa")
                rS = tpool.tile([C, 128], F32, tag="rS")
                nc.vector.reciprocal(rS[:], S_t[:])
                aggN = tpool.tile([C, 128], F32, tag="aggN")
                nc.vector.tensor_tensor(aggN[:], aggU[:], rS[:], MULT)
                l1 = tpool.tile([C, 128], F32, tag="l1")
                nc.vector.scalar_tensor_tensor(l1[:], aggN[:], bvp[:],
                                               ftw[:, bass.ts(t, 128)], ADD, ADD)
                outc = tpool.tile([C, 128], F32, tag="outc")
                nc.vector.scalar_tensor_tensor(outc[:], l1[:], 0.1, l1[:],
                                               MULT, MAX)
                nc.sync.dma_start(outT[:, bass.ts(t, 128)], outc[:])

            for i in range(n_tiles + 3):
                if i < n_tiles:
                    s0_gather(i)
                if 1 <= i < n_tiles + 1:
                    s1_front(i - 1)
                if 2 <= i < n_tiles + 2:
                    s2_chunks(i - 2)
                if 3 <= i:
                    s3_tail(i - 3)

    from concourse.library_overlay import lower_extended_insts
    lower_extended_insts(nc)
    if split_waits:
        split_excess_waits(nc)
    return nc




# ------------------------------------------------------------- entry point
N_CORES = 8
THRESH = 32767  # int16 row-index reach (with +1 zero-row offset)

_CACHE = {}


def kernel(**inputs) -> np.ndarray:
    """Full-input entry: shards points across 8 NeuronCores, runs the Bass
    kernel via run_bass_kernel_spmd, reassembles the full (1, N, C) output."""
    from concourse.bass_utils import run_bass_kernel_spmd

    xyz = np.asarray(inputs["xyz"], np.float32)
    feats = np.asarray(inputs["feats"], np.float32)
    nei = np.asarray(inputs["nei_ind"])
    params = {k: np.asarray(v, np.float32) for k, v in inputs.items()
              if k not in ("xyz", "feats", "nei_ind")}

    in_maps, meta = prep_inputs(xyz, feats, nei, params, N_CORES, THRESH)

    key = (meta["n_tiles"], meta["per_core"], meta["lo_rows"], meta["hi_rows"])
    if key not in _CACHE:
        _CACHE[key] = build_nc(meta)
    nc = _CACHE[key]

    res = run_bass_kernel_spmd(nc, in_maps, core_ids=list(range(N_CORES)))
    outs = [r["outT"] for r in res.results]          # each [C, per_core] f32
    full = np.concatenate(outs, axis=1).T             # [npad, C]
    return np.ascontiguousarray(full[None, :meta["n_real"]]).astype(np.float32)



# revision 33
# speedup vs baseline: 1.3214x; 1.3214x over previous
"""PointTransformerLayer Bass kernel for TRN2 (v2).

Per-core design (points sharded 8 ways, table replicated):
  - DRAM gather table: one 256B row per point = 128 fp8(e4m3) feats packed
    as channel pairs (u16 word w = channels 2w, 2w+1).  Gathered in
    transpose mode the tile is natively in fp8-DoubleRow moving-operand
    layout, so Wk/Wv matmuls run at 0.5 cyc/col.
  - int16 index reach is 32767 rows, so the table is split lo/hi with a
    zero row at index 0.  Per point the 16 neighbor slots are reordered
    (softmax over K is permutation invariant) so hi-table refs occupy the
    top k-blocks; points are sorted per core by hi-count so each 128-pt
    tile needs only hi_n[t] = 128*max_hi descriptors for the hi gather.
    No on-chip merge: lo and hi gathered tiles are separately accumulated
    into PSUM by the (linear) Wk/Wv matmuls; missing slots fetch zero rows.
  - Pair order within a tile is k-major: col = 128*k + pt.
  - The position term relu(a_nbr - a_ctr) is precomputed on host and
    streamed as a [2,2,N] fp8 DoubleRow operand (3 channels + ones; the
    ones row carries nothing for w (bias via ACT) and 16*bvp for v).
  - All fp8 weights carry a x16 scale; undone via ACT scale (w path),
    exp bias (e path) and reciprocal scale (normalization).
  - Back-end packing: h = relu(Ww1'r) lands as [128,128] PSUM (8 matmuls
    with partition offsets), Ww2 is applied block-diagonally (128 cols),
    exp runs on [128,128], softmax denom comes from a one-hot reduction
    matmul accumulated across chunks, and e is re-broadcast to [128,1024]
    PSUM by 8 one-hot matmuls.
  - t2 = e*v on DVE straight from PSUM; K-reduction = 3-level strided
    tree adds in f16.
"""

import sys

sys.path.insert(0, "/opt/trn_rl_repo")
sys.path.insert(0, "/root/.axon_site/_ro/trn_rl_repo")

import numpy as np
import ml_dtypes

import concourse.bass as bass
import concourse.tile as tile
from concourse import mybir

F16 = mybir.dt.float16
F32 = mybir.dt.float32
FP8 = mybir.dt.float8e4
I16 = mybir.dt.int16
NPF8 = ml_dtypes.float8_e4m3

K = 16
C = 128
S = 8
CS = C // S  # 16
EPS = 1e-5
EXP_SHIFT = float(np.log(256.0))
SCALE = 16.0
LN_SCALE = float(np.log(SCALE))
PT_TILE = 128
NPAIR = PT_TILE * K     # 2048
CHUNK = 1024
MM = 512
NBLK = NPAIR // PT_TILE  # 16 k-blocks per tile
THRESH = 32767           # entries in lo table (idx = e+1 <= 32767)

N_CORES = 8


# ----------------------------------------------------------------- host math
def fold_params(p):
    f32 = np.float32
    s_p = (p["p_gamma"] / np.sqrt(p["p_var"] + EPS)).astype(f32)
    Afold = (p["Wp1"] * s_p[None, :]).astype(f32)
    cfold = ((p["bp1"] - p["p_mean"]) * s_p + p["p_beta"]).astype(f32)

    s_w = (p["w_gamma"] / np.sqrt(p["w_var"] + EPS)).astype(f32)
    ball = (p["bk"] - p["bq"] + p["bp2"]).astype(f32)
    b_w = ((ball - p["w_mean"]) * s_w + p["w_beta"]).astype(f32)

    s1 = (p["w1_gamma"] / np.sqrt(p["w1_var"] + EPS)).astype(f32)
    ww1s = (p["Ww1"] * s1[None, :]).astype(np.float16)        # [128, 16]
    b1f = ((p["bw1"] - p["w1_mean"]) * s1 + p["w1_beta"]).astype(f32)

    bvp = (p["bv"] + p["bp2"]).astype(f32)                    # [128]

    def pack_dr(w):  # [in_ch, out] f32 -> [in_ch//2, 2, out] fp8
        ic = w.shape[0]
        return np.ascontiguousarray(
            w.reshape(ic // 2, 2, w.shape[1])).astype(NPF8)

    wk_dr = pack_dr(SCALE * s_w[None, :] * p["Wk"])           # [64,2,128]
    wv_f16 = (SCALE * p["Wv"]).astype(np.float16)             # [128,128]
    wq_f16 = (-SCALE * s_w[None, :] * p["Wq"]).astype(np.float16)  # [128,128]

    # ru4 channels: 0..2 = relu(u), 3 = ones
    wp2w = np.zeros((4, C), np.float32)
    wp2w[:3] = SCALE * s_w[None, :] * p["Wp2"]
    wp2w_f16 = wp2w.astype(np.float16)                        # [4,128]
    wp2v = np.zeros((4, C), np.float32)
    wp2v[:3] = SCALE * p["Wp2"]
    wp2v[3] = SCALE * bvp
    wp2v_f16 = wp2v.astype(np.float16)                        # [4,128]

    # packed h layout: 4 groups of 32 partitions (16 real + 16 zero hole);
    # group g covers pair-columns [256g, 256g+256) of a 1024-col chunk.
    ww1z = np.zeros((C, 32), np.float16)       # Ww1 + 16 zero out-cols
    ww1z[:, :16] = ww1s
    b1f_pk = np.zeros(C, f32)
    for g in range(4):
        b1f_pk[32 * g:32 * g + 16] = b1f
    # replicated-logits stationaries: block g maps packed h (partitions
    # 32g+i) to all 128 replicated logit channels for its 256 columns
    w2r = np.zeros((C, 4 * C), np.float16)
    for g in range(4):
        for c in range(C):
            for i in range(16):
                w2r[32 * g + i, g * C + c] = p["Ww2"][i, c % 16]
    be_rep = (np.array([p["bw2"][c % 16] for c in range(C)], f32)
              - EXP_SHIFT - LN_SCALE)

    return dict(
        Afold=Afold, cfold=cfold, b_w=b_w, ww1z=ww1z,
        wk_dr=wk_dr, wv_f16=wv_f16, wq_f16=wq_f16,
        wp2w_f16=wp2w_f16, wp2v_f16=wp2v_f16,
        be_rep=be_rep, b1f_pk=b1f_pk, w2r=w2r,
    )


def prep_inputs(xyz, feats, nei_ind, params, n_cores):
    f = fold_params(params)
    n_real = feats.shape[1]
    per_core_raw = -(-n_real // n_cores)
    per_core = -(-per_core_raw // PT_TILE) * PT_TILE
    npad = per_core * n_cores
    n_tiles = per_core // PT_TILE

    feats0 = np.zeros((npad, C), np.float32)
    feats0[:n_real] = feats[0]
    pos0 = np.zeros((npad, 3), np.float32)
    pos0[:n_real] = xyz[0]
    ni = np.zeros((npad, K), np.int64)
    ni[:n_real] = nei_ind[0]

    a = (pos0 @ f["Afold"]).astype(np.float32)            # [npad, 3]
    actr = (a - f["cfold"][None, :]).astype(np.float32)   # center role

    # table rows (512B): words 0..63 = fp8 channel-paired feats (k path),
    # words 128..255 = f16 feats (v path)
    feats8 = feats0.astype(NPF8)                          # [npad, 128] fp8
    ent = np.zeros((npad, 256), np.uint16)
    ent[:, :64] = feats8.view(np.uint8)[:, 0::2].astype(np.uint16) \
        | (feats8.view(np.uint8)[:, 1::2].astype(np.uint16) << 8)
    ent[:, 128:256] = feats0.astype(np.float16).view(np.uint16)
    lo_rows = THRESH + 1
    hi_rows = npad - THRESH + 1
    table_lo = np.zeros((lo_rows, 256), np.uint16)
    table_lo[1:] = ent[:THRESH]
    table_hi = np.zeros((hi_rows, 256), np.uint16)
    table_hi[1:] = ent[THRESH:]
    table_lo = table_lo.view(np.float16)
    table_hi = table_hi.view(np.float16)

    # per-point slot sort: lo refs first, hi refs last
    is_hi = ni >= THRESH                                  # [npad, K]
    slot_order = np.argsort(is_hi, axis=1, kind="stable") # [npad, K]
    ni_sorted = np.take_along_axis(ni, slot_order, axis=1)
    hi_cnt = is_hi.sum(axis=1)                            # [npad]

    def wrap(arr):
        # arr [n] -> [128, n//16] int16 (16-wrap replicated to 128)
        n = arr.shape[0]
        w16 = arr.reshape(n // 16, 16).T.astype(np.int16)
        return np.tile(w16, (8, 1))

    in_maps = []
    metas = []
    for cidx in range(n_cores):
        sl = slice(cidx * per_core, (cidx + 1) * per_core)
        # sort this core's points by hi-count so tiles are homogeneous
        hc = hi_cnt[sl]
        perm = np.argsort(hc, kind="stable")              # local indices
        gperm = cidx * per_core + perm                    # global ids
        nis = ni_sorted[gperm]                            # [per_core, K]

        hi_n = []
        lo_cols = np.zeros((128, n_tiles * 128), np.int16)
        ent_core = np.zeros((n_tiles, NPAIR), np.int64)
        ru_blocks = np.zeros((4, n_tiles * NPAIR), np.float16)
        for t in range(n_tiles):
            pts = nis[t * PT_TILE:(t + 1) * PT_TILE]      # [128, K]
            # k-major pair order: col j = 128*k + pt -> entry pts[pt, k]
            ent_t = pts.T.reshape(-1)                     # [2048] (k-major)
            ent_core[t] = ent_t
            lo_v = np.where(ent_t < THRESH, ent_t + 1, 0).astype(np.int16)
            lo_cols[:, t * 128:(t + 1) * 128] = wrap(lo_v)
            mh = int(hi_cnt[gperm[t * PT_TILE:(t + 1) * PT_TILE]].max())
            hi_n.append(128 * mh)
            # ru for this tile, k-major
            ctr = np.repeat(gperm[t * PT_TILE:(t + 1) * PT_TILE][None, :],
                            K, axis=0).reshape(-1)        # [2048]
            u = a[ent_t] - actr[ctr]                      # [2048, 3]
            ru = np.maximum(u, 0.0).astype(np.float32)
            ru4 = np.concatenate([ru, np.ones((NPAIR, 1), np.float32)],
                                 axis=1)                  # [2048, 4]
            ru_blocks[:, t * NPAIR:(t + 1) * NPAIR] = \
                ru4.T.astype(np.float16)

        ftw = np.ascontiguousarray(feats0[gperm].T.astype(np.float16))

        in_maps.append({
            "table_lo": table_lo, "table_hi": table_hi,
            "idx_lo": lo_cols,
            "ru4": ru_blocks, "ftw": ftw,
            "wk_dr": f["wk_dr"], "wv_f16": f["wv_f16"],
            "wq_f16": f["wq_f16"],
            "wp2w_f16": f["wp2w_f16"], "wp2v_f16": f["wp2v_f16"],
            "ww1z": f["ww1z"], "w2r": f["w2r"],
            "b_w": f["b_w"].reshape(C, 1),
            "b1f_pk": f["b1f_pk"].reshape(C, 1),
            "be_rep": f["be_rep"].reshape(C, 1),
        })
        metas.append(dict(hi_n=tuple(hi_n), perm=perm, ent=ent_core,
                          wrap=wrap))

    meta = dict(n_tiles=n_tiles, per_core=per_core, npad=npad,
                lo_rows=lo_rows, hi_rows=hi_rows, n_real=n_real,
                hi_ns=tuple(m["hi_n"] for m in metas),
                perms=[m["perm"] for m in metas],
                ents=[m["ent"] for m in metas], wrap=wrap)
    return in_maps, meta


# ------------------------------------------------------------- walrus compat
def split_excess_waits(nc, max_waits=1):
    n_split = 0
    for fn in nc.m.functions:
        for blk in fn.blocks:
            new_insts = []
            for inst in blk.instructions:
                si = inst.sync_info
                lim = (1 if isinstance(inst, (mybir.InstDrain, mybir.InstNoOp,
                                              mybir.InstEventSemaphore))
                       else max_waits)
                if si is not None and si.on_wait and len(si.on_wait) > lim:
                    waits = list(si.on_wait)
                    extra, keep = waits[:-lim], waits[-lim:]
                    ci = 0
                    while extra:
                        chunk, extra = extra[:1], extra[1:]
                        new_insts.append(mybir.InstNoOp(
                            name=f"{inst.name}-waitsplit{ci}",
                            engine=inst.engine,
                            bass_nofuse=True,
                            sync_info=mybir.SyncInfo(on_wait=chunk, on_update=[]),
                        ))
                        ci += 1
                    si.on_wait = keep
                    n_split += 1
                new_insts.append(inst)
            blk.instructions = new_insts
    return n_split


# ----------------------------------------------------------------- the kernel
def build_nc(meta, hi_n):
    from concourse import library_config
    n_tiles = meta["n_tiles"]
    per_core = meta["per_core"]
    hi_tot = sum(hi_n)
    hi_off = np.concatenate([[0], np.cumsum(hi_n)]).astype(int)
    nc = bass.Bass("TRN2", target_bir_lowering=False, debug=False,
                   num_swdge_queues=1)

    dt_ = nc.dram_tensor
    t_lo = dt_("table_lo", [meta["lo_rows"], 256], F16, kind="ExternalInput").ap()
    t_hi = dt_("table_hi", [meta["hi_rows"], 256], F16, kind="ExternalInput").ap()
    idx_lo = dt_("idx_lo", [128, n_tiles * 128], I16, kind="ExternalInput").ap()
    idx_hi_cols = max(hi_tot // 16, 8)
    idx_hi = dt_("idx_hi", [128, idx_hi_cols], I16, kind="ExternalInput").ap()
    ru4_d = dt_("ru4", [4, n_tiles * NPAIR], F16, kind="ExternalInput").ap()
    ftw_d = dt_("ftw", [C, per_core], F16, kind="ExternalInput").ap()
    wk_d = dt_("wk_dr", [64, 2, C], FP8, kind="ExternalInput").ap()
    wv_d = dt_("wv_f16", [C, C], F16, kind="ExternalInput").ap()
    wq_d = dt_("wq_f16", [C, C], F16, kind="ExternalInput").ap()
    wp2w_d = dt_("wp2w_f16", [4, C], F16, kind="ExternalInput").ap()
    wp2v_d = dt_("wp2v_f16", [4, C], F16, kind="ExternalInput").ap()
    ww1z_d = dt_("ww1z", [C, 32], F16, kind="ExternalInput").ap()
    w2r_d = dt_("w2r", [C, 4 * C], F16, kind="ExternalInput").ap()
    b_w_d = dt_("b_w", [C, 1], F32, kind="ExternalInput").ap()
    b1f_d = dt_("b1f_pk", [C, 1], F32, kind="ExternalInput").ap()
    be_d = dt_("be_rep", [C, 1], F32, kind="ExternalInput").ap()
    outT = dt_("outT", [C, per_core], F32, kind="ExternalOutput").ap()

    Relu = mybir.ActivationFunctionType.Relu
    Exp = mybir.ActivationFunctionType.Exp
    DR = mybir.MatmulPerfMode.DoubleRow
    ADD = mybir.AluOpType.add
    MULT = mybir.AluOpType.mult
    MAX = mybir.AluOpType.max

    nc.gpsimd.load_library(library_config.mlp)
    rlo = nc.gpsimd.alloc_register("nlo")
    nc.gpsimd.reg_mov(rlo, NPAIR)
    rhi = nc.gpsimd.alloc_register("nhi")

    with tile.TileContext(nc) as tc:
        with (
            tc.tile_pool(name="const", bufs=1) as cpool,
            tc.tile_pool(name="gath", bufs=2) as gpool,
            tc.tile_pool(name="ru", bufs=2) as rupool,
            tc.tile_pool(name="work", bufs=2) as wpool,
            tc.tile_pool(name="tail", bufs=2) as tpool,
            tc.tile_pool(name="psW", bufs=1, space="PSUM") as psW,
            tc.tile_pool(name="psV", bufs=1, space="PSUM") as psV,
            tc.tile_pool(name="psX", bufs=2, space="PSUM") as psX,
        ):
            def cload(ap_dram, shape, dtype, tag):
                t = cpool.tile(shape, dtype, tag=tag)
                nc.sync.dma_start(t[:], ap_dram)
                return t

            wk = cload(wk_d, [64, 2, C], FP8, "wk")
            wv = cload(wv_d, [C, C], F16, "wv")
            wq = cload(wq_d, [C, C], F16, "wq")
            wp2w = cload(wp2w_d, [4, C], F16, "wp2w")
            wp2v = cload(wp2v_d, [4, C], F16, "wp2v")
            ww1z = cload(ww1z_d, [C, 32], F16, "ww1z")
            w2r = cload(w2r_d, [C, 4 * C], F16, "w2r")
            b_w = cload(b_w_d, [C, 1], F32, "b_w")
            b1f = cload(b1f_d, [C, 1], F32, "b1f")
            be_b = cload(be_d, [C, 1], F32, "be_b")
            ixlo = cload(idx_lo, [128, n_tiles * 128], I16, "ixlo")
            ixhi = cload(idx_hi, [128, idx_hi_cols], I16, "ixhi")
            ftw = cpool.tile([C, per_core], F16, tag="ftw")
            nc.sync.dma_start(ftw[:], ftw_d)

            state = {}

            def s0_gather(t):
                glo = gpool.tile([128, 2, NPAIR], F16, tag="glo")
                nc.gpsimd.dma_gather(glo[:], t_lo,
                                     ixlo[:, bass.ts(t, 128)], NPAIR, rlo,
                                     256, transpose=True, queue_num=0,
                                     single_packet=False)
                hn = hi_n[t]
                ghi = None
                if hn > 0:
                    ghi = gpool.tile([128, 2, hn], F16, tag="ghi")
                    nc.gpsimd.reg_mov(rhi, hn)
                    nc.gpsimd.dma_gather(
                        ghi[:], t_hi,
                        ixhi[:, hi_off[t] // 16:hi_off[t + 1] // 16], hn, rhi,
                        256, transpose=True, queue_num=0,
                        single_packet=False)
                ru4 = rupool.tile([4, NPAIR], F16, tag="ru4")
                nc.sync.dma_start(ru4[:],
                                  ru4_d[:, t * NPAIR:(t + 1) * NPAIR])
                state[("g", t)] = (glo, ghi, ru4)

            def s1_tile(t):
                glo, ghi, ru4 = state.pop(("g", t))
                hn = hi_n[t]
                # k-path fp8 channel-pair views + v-path f16 views
                glo8 = glo[0:64, 0, :].bitcast(FP8)   # [64, 2*NPAIR]
                glo16 = glo[:, 1, :]                  # [128, NPAIR] f16
                ghi8 = ghi[0:64, 0, :].bitcast(FP8) if hn else None
                ghi16 = ghi[:, 1, :] if hn else None

                aggP = []
                SP = []

                for ch in range(NPAIR // CHUNK):
                    c0 = ch * CHUNK
                    wps = psW.tile([C, CHUNK], F32, tag="wps")
                    # --- wps = scaled (kf - q + p) accumulation
                    for hf in range(CHUNK // MM):
                        lo = c0 + hf * MM
                        o = wps[:, hf * MM:(hf + 1) * MM]
                        nc.tensor.matmul(
                            o, wk[:],
                            glo8[:, 2 * lo:2 * (lo + MM)]
                            .rearrange("p (n two) -> p two n", two=2),
                            start=True, stop=False, perf_mode=DR)
                        # q broadcast: col 128k+pt -> ftw col pt
                        qv = (ftw[:, t * PT_TILE:(t + 1) * PT_TILE]
                              .unsqueeze(1).broadcast_to([C, MM // PT_TILE,
                                                          PT_TILE]))
                        nc.tensor.matmul(o, wq[:], qv, start=False, stop=False)
                        nc.tensor.matmul(
                            o, wp2w[:], ru4[:, lo:lo + MM],
                            start=False, stop=(hn == 0 or lo + MM <= NPAIR - hn))
                        if hn and lo + MM > NPAIR - hn:
                            ho = max(NPAIR - hn, lo)
                            hb = NPAIR - hn
                            nc.tensor.matmul(
                                wps[:, ho - c0:(hf + 1) * MM], wk[:],
                                ghi8[:, 2 * (ho - hb):2 * (lo + MM - hb)]
                                .rearrange("p (n two) -> p two n", two=2),
                                start=False, stop=True, perf_mode=DR)
                    r_t = wpool.tile([C, CHUNK], F16, tag="r")
                    nc.scalar.activation(r_t[:], wps[:], Relu,
                                         bias=b_w[:], scale=1.0 / SCALE)
                    # --- h packed [128, 256]: 4 groups of 256 cols at
                    # 32-aligned partition offsets (16 real + 16 zero rows)
                    hps = psX.tile([C, 2 * PT_TILE], F32, tag="scr")
                    for g in range(4):
                        nc.tensor.matmul(
                            hps[32 * g:32 * g + 32, :], ww1z[:],
                            r_t[:, 2 * g * PT_TILE:2 * (g + 1) * PT_TILE],
                            start=True, stop=True, tile_position=(0, 32 * g))
                    h2 = wpool.tile([C, 2 * PT_TILE], F16, tag="h2")
                    nc.scalar.activation(h2[:], hps[:], Relu, bias=b1f[:])
                    # --- replicated logits [128, CHUNK] from packed h
                    lps = psX.tile([C, CHUNK], F32, tag="scr")
                    for g in range(4):
                        nc.tensor.matmul(
                            lps[:, 2 * g * PT_TILE:2 * (g + 1) * PT_TILE],
                            w2r[:, g * C:(g + 1) * C], h2[:],
                            start=True, stop=True)
                    e_sb = wpool.tile([C, CHUNK], F16, tag="e_sb")
                    nc.scalar.activation(e_sb[:], lps[:], Exp, bias=be_b[:])
                    # --- softmax denominator tree for this chunk
                    ev = e_sb[:].rearrange("p (a b) -> p a b", b=PT_TILE)
                    s4 = wpool.tile([C, 4 * PT_TILE], F16, tag="s4")
                    s4v = s4[:].rearrange("p (a b) -> p a b", b=PT_TILE)
                    nc.vector.tensor_tensor(s4v, ev[:, 0:4], ev[:, 4:8], ADD)
                    s8 = wpool.tile([C, 2 * PT_TILE], F16, tag="s8")
                    s8v = s8[:].rearrange("p (a b) -> p a b", b=PT_TILE)
                    nc.vector.tensor_tensor(s8v, s4v[:, 0:2], s4v[:, 2:4], ADD)
                    sp = tpool.tile([C, PT_TILE], F16, tag=f"SP{ch}")
                    nc.vector.tensor_tensor(sp[:], s8v[:, 0], s8v[:, 1], ADD)
                    SP.append(sp)
                    # --- v path
                    vps = psV.tile([C, CHUNK], F32, tag="vps")
                    for hf in range(CHUNK // MM):
                        lo = c0 + hf * MM
                        o = vps[:, hf * MM:(hf + 1) * MM]
                        nc.tensor.matmul(o, wv[:], glo16[:, lo:lo + MM],
                                         start=True, stop=False)
                        nc.tensor.matmul(
                            o, wp2v[:], ru4[:, lo:lo + MM],
                            start=False, stop=(hn == 0 or lo + MM <= NPAIR - hn))
                        if hn and lo + MM > NPAIR - hn:
                            ho = max(NPAIR - hn, lo)
                            hb = NPAIR - hn
                            nc.tensor.matmul(
                                vps[:, ho - c0:(hf + 1) * MM], wv[:],
                                ghi16[:, ho - hb:lo + MM - hb],
                                start=False, stop=True)
                    # --- t2 = e * v and K-tree
                    t2 = wpool.tile([C, CHUNK], F16, tag="t2")
                    nc.vector.tensor_tensor(t2[:], e_sb[:], vps[:], MULT)
                    t2v = t2[:].rearrange("p (a b) -> p a b", b=PT_TILE)
                    t4 = wpool.tile([C, 4 * PT_TILE], F16, tag="t4")
                    t4v = t4[:].rearrange("p (a b) -> p a b", b=PT_TILE)
                    nc.vector.tensor_tensor(t4v, t2v[:, 0:4], t2v[:, 4:8], ADD)
                    t8 = wpool.tile([C, 2 * PT_TILE], F16, tag="t8")
                    t8v = t8[:].rearrange("p (a b) -> p a b", b=PT_TILE)
                    nc.vector.tensor_tensor(t8v, t4v[:, 0:2], t4v[:, 2:4], ADD)
                    ap = tpool.tile([C, PT_TILE], F16, tag=f"aggP{ch}")
                    nc.vector.tensor_tensor(ap[:], t8v[:, 0], t8v[:, 1], ADD)
                    aggP.append(ap)

                # ---- tail
                aggU = tpool.tile([C, PT_TILE], F16, tag="aggU")
                nc.vector.tensor_tensor(aggU[:], aggP[0][:], aggP[1][:], ADD)
                S16 = tpool.tile([C, PT_TILE], F16, tag="S16")
                nc.vector.tensor_tensor(S16[:], SP[0][:], SP[1][:], ADD)
                rS = tpool.tile([C, PT_TILE], F16, tag="rS")
                with nc.allow_low_precision("softmax denom recip in f16"):
                    nc.vector.reciprocal(rS[:], S16[:])
                # aggN = aggU * rS / SCALE  (rS is 1/(S_true/SCALE))
                aggN = tpool.tile([C, PT_TILE], F16, tag="aggN")
                nc.vector.scalar_tensor_tensor(aggN[:], aggU[:], 1.0 / SCALE,
                                               rS[:], MULT, MULT)
                l2 = tpool.tile([C, PT_TILE], F16, tag="l2")
                nc.vector.tensor_tensor(l2[:], aggN[:],
                                        ftw[:, bass.ts(t, PT_TILE)], ADD)
                outc = tpool.tile([C, PT_TILE], F32, tag="outc")
                nc.vector.scalar_tensor_tensor(outc[:], l2[:], 0.1, l2[:],
                                               MULT, MAX)
                nc.sync.dma_start(outT[:, bass.ts(t, PT_TILE)], outc[:])

            for i in range(n_tiles + 1):
                if i < n_tiles:
                    s0_gather(i)
                if i >= 1:
                    s1_tile(i - 1)

    from concourse.library_overlay import lower_extended_insts
    lower_extended_insts(nc)
    split_excess_waits(nc)
    return nc


# ------------------------------------------------------------- entry point
_CACHE = {}


def kernel(**inputs) -> np.ndarray:
    from concourse.bass_utils import run_bass_kernel_spmd

    xyz = np.asarray(inputs["xyz"], np.float32)
    feats = np.asarray(inputs["feats"], np.float32)
    nei = np.asarray(inputs["nei_ind"])
    params = {k: np.asarray(v, np.float32) for k, v in inputs.items()
              if k not in ("xyz", "feats", "nei_ind")}

    in_maps, meta = prep_inputs(xyz, feats, nei, params, N_CORES)

    # one compiled program per distinct hi_n profile; all cores share the
    # max profile so a single SPMD binary serves all 8
    hi_max = tuple(int(max(meta["hi_ns"][c][t] for c in range(N_CORES)))
                   for t in range(meta["n_tiles"]))
    key = (meta["n_tiles"], meta["per_core"], hi_max)
    if key not in _CACHE:
        _CACHE[key] = build_nc(meta, hi_max)
    nc = _CACHE[key]

    # build each core's hi idx stream against the shared profile
    hi_off_shared = np.concatenate([[0], np.cumsum(hi_max)]).astype(int)
    tot = max(int(hi_off_shared[-1]) // 16, 8)
    for cidx in range(N_CORES):
        dst = np.zeros((128, tot), np.int16)
        ent_core = meta["ents"][cidx]
        for t in range(meta["n_tiles"]):
            hn = hi_max[t]
            if hn:
                hv = ent_core[t][NPAIR - hn:]
                hv = np.where(hv >= THRESH, hv - THRESH + 1, 0
                              ).astype(np.int16)
                dst[:, hi_off_shared[t] // 16:hi_off_shared[t + 1] // 16] = \
                    meta["wrap"](hv)
        in_maps[cidx]["idx_hi"] = dst

    res = run_bass_kernel_spmd(nc, in_maps, core_ids=list(range(N_CORES)))
    outs = []
    for cidx, r in enumerate(res.results):
        o = np.asarray(r["outT"]).T                      # [per_core, C]
        inv = np.empty_like(meta["perms"][cidx])
        inv[meta["perms"][cidx]] = np.arange(len(inv))
        outs.append(o[inv])
    full = np.concatenate(outs, axis=0)                  # [npad, C]
    return np.ascontiguousarray(full[None, :meta["n_real"]]).astype(np.float32)


# revision 38
# speedup vs baseline: 1.3240x; 1.0020x over previous
"""PointTransformerLayer Bass kernel for TRN2 (v2).

Per-core design (points sharded 8 ways, table replicated):
  - DRAM gather table: one 256B row per point = 128 fp8(e4m3) feats packed
    as channel pairs (u16 word w = channels 2w, 2w+1).  Gathered in
    transpose mode the tile is natively in fp8-DoubleRow moving-operand
    layout, so Wk/Wv matmuls run at 0.5 cyc/col.
  - int16 index reach is 32767 rows, so the table is split lo/hi with a
    zero row at index 0.  Per point the 16 neighbor slots are reordered
    (softmax over K is permutation invariant) so hi-table refs occupy the
    top k-blocks; points are sorted per core by hi-count so each 128-pt
    tile needs only hi_n[t] = 128*max_hi descriptors for the hi gather.
    No on-chip merge: lo and hi gathered tiles are separately accumulated
    into PSUM by the (linear) Wk/Wv matmuls; missing slots fetch zero rows.
  - Pair order within a tile is k-major: col = 128*k + pt.
  - The position term relu(a_nbr - a_ctr) is precomputed on host and
    streamed as a [2,2,N] fp8 DoubleRow operand (3 channels + ones; the
    ones row carries nothing for w (bias via ACT) and 16*bvp for v).
  - All fp8 weights carry a x16 scale; undone via ACT scale (w path),
    exp bias (e path) and reciprocal scale (normalization).
  - Back-end packing: h = relu(Ww1'r) lands as [128,128] PSUM (8 matmuls
    with partition offsets), Ww2 is applied block-diagonally (128 cols),
    exp runs on [128,128], softmax denom comes from a one-hot reduction
    matmul accumulated across chunks, and e is re-broadcast to [128,1024]
    PSUM by 8 one-hot matmuls.
  - t2 = e*v on DVE straight from PSUM; K-reduction = 3-level strided
    tree adds in f16.
"""

import sys

sys.path.insert(0, "/opt/trn_rl_repo")
sys.path.insert(0, "/root/.axon_site/_ro/trn_rl_repo")

import numpy as np
import ml_dtypes

import concourse.bass as bass
import concourse.tile as tile
from concourse import mybir

F16 = mybir.dt.float16
F32 = mybir.dt.float32
FP8 = mybir.dt.float8e4
I16 = mybir.dt.int16
NPF8 = ml_dtypes.float8_e4m3

K = 16
C = 128
S = 8
CS = C // S  # 16
EPS = 1e-5
EXP_SHIFT = float(np.log(256.0))
SCALE = 16.0
LN_SCALE = float(np.log(SCALE))
PT_TILE = 128
NPAIR = PT_TILE * K     # 2048
CHUNK = 1024
MM = 512
NBLK = NPAIR // PT_TILE  # 16 k-blocks per tile
THRESH = 32767           # entries in lo table (idx = e+1 <= 32767)

N_CORES = 8


# ----------------------------------------------------------------- host math
def fold_params(p):
    f32 = np.float32
    s_p = (p["p_gamma"] / np.sqrt(p["p_var"] + EPS)).astype(f32)
    Afold = (p["Wp1"] * s_p[None, :]).astype(f32)
    cfold = ((p["bp1"] - p["p_mean"]) * s_p + p["p_beta"]).astype(f32)

    s_w = (p["w_gamma"] / np.sqrt(p["w_var"] + EPS)).astype(f32)
    ball = (p["bk"] - p["bq"] + p["bp2"]).astype(f32)
    b_w = ((ball - p["w_mean"]) * s_w + p["w_beta"]).astype(f32)

    s1 = (p["w1_gamma"] / np.sqrt(p["w1_var"] + EPS)).astype(f32)
    ww1s = (p["Ww1"] * s1[None, :]).astype(np.float16)        # [128, 16]
    b1f = ((p["bw1"] - p["w1_mean"]) * s1 + p["w1_beta"]).astype(f32)

    bvp = (p["bv"] + p["bp2"]).astype(f32)                    # [128]

    def pack_dr(w):  # [in_ch, out] f32 -> [in_ch//2, 2, out] fp8
        ic = w.shape[0]
        return np.ascontiguousarray(
            w.reshape(ic // 2, 2, w.shape[1])).astype(NPF8)

    wk_dr = pack_dr(SCALE * s_w[None, :] * p["Wk"])           # [64,2,128]
    wv_f16 = (SCALE * p["Wv"]).astype(np.float16)             # [128,128]
    wq_f16 = (-SCALE * s_w[None, :] * p["Wq"]).astype(np.float16)  # [128,128]

    # ru4 channels: 0..2 = relu(u), 3 = ones
    wp2w = np.zeros((4, C), np.float32)
    wp2w[:3] = SCALE * s_w[None, :] * p["Wp2"]
    wp2w_dr = pack_dr(wp2w)                                   # [2,2,128]
    wp2v = np.zeros((4, C), np.float32)
    wp2v[:3] = SCALE * p["Wp2"]
    wp2v[3] = SCALE * bvp
    wp2v_f16 = wp2v.astype(np.float16)                        # [4,128]

    # packed h layout: 4 groups of 32 partitions (16 real + 16 zero hole);
    # group g covers pair-columns [256g, 256g+256) of a 1024-col chunk.
    ww1z = np.zeros((C, 32), np.float16)       # Ww1 + 16 zero out-cols
    ww1z[:, :16] = ww1s
    b1f_pk = np.zeros(C, f32)
    for g in range(4):
        b1f_pk[32 * g:32 * g + 16] = b1f
    # replicated-logits stationaries: block g maps packed h (partitions
    # 32g+i) to all 128 replicated logit channels for its 256 columns
    w2r = np.zeros((C, 4 * C), np.float16)
    for g in range(4):
        for c in range(C):
            for i in range(16):
                w2r[32 * g + i, g * C + c] = p["Ww2"][i, c % 16]
    be_rep = (np.array([p["bw2"][c % 16] for c in range(C)], f32)
              - EXP_SHIFT - LN_SCALE)

    return dict(
        Afold=Afold, cfold=cfold, b_w=b_w, ww1z=ww1z,
        wk_dr=wk_dr, wv_f16=wv_f16, wq_f16=wq_f16,
        wp2w_dr=wp2w_dr, wp2v_f16=wp2v_f16,
        be_rep=be_rep, b1f_pk=b1f_pk, w2r=w2r,
    )


def prep_inputs(xyz, feats, nei_ind, params, n_cores):
    f = fold_params(params)
    n_real = feats.shape[1]
    per_core_raw = -(-n_real // n_cores)
    per_core = -(-per_core_raw // PT_TILE) * PT_TILE
    npad = per_core * n_cores
    n_tiles = per_core // PT_TILE

    feats0 = np.zeros((npad, C), np.float32)
    feats0[:n_real] = feats[0]
    pos0 = np.zeros((npad, 3), np.float32)
    pos0[:n_real] = xyz[0]
    ni = np.zeros((npad, K), np.int64)
    ni[:n_real] = nei_ind[0]

    a = (pos0 @ f["Afold"]).astype(np.float32)            # [npad, 3]
    actr = (a - f["cfold"][None, :]).astype(np.float32)   # center role

    # table rows (512B): words 0..63 = fp8 channel-paired feats (k path),
    # words 128..255 = f16 feats (v path)
    feats8 = feats0.astype(NPF8)                          # [npad, 128] fp8
    ent = np.zeros((npad, 256), np.uint16)
    ent[:, :64] = feats8.view(np.uint8)[:, 0::2].astype(np.uint16) \
        | (feats8.view(np.uint8)[:, 1::2].astype(np.uint16) << 8)
    ent[:, 128:256] = feats0.astype(np.float16).view(np.uint16)
    lo_rows = THRESH + 1
    hi_rows = npad - THRESH + 1
    table_lo = np.zeros((lo_rows, 256), np.uint16)
    table_lo[1:] = ent[:THRESH]
    table_hi = np.zeros((hi_rows, 256), np.uint16)
    table_hi[1:] = ent[THRESH:]
    table_lo = table_lo.view(np.float16)
    table_hi = table_hi.view(np.float16)

    # per-point slot sort: lo refs first, hi refs last
    is_hi = ni >= THRESH                                  # [npad, K]
    slot_order = np.argsort(is_hi, axis=1, kind="stable") # [npad, K]
    ni_sorted = np.take_along_axis(ni, slot_order, axis=1)
    hi_cnt = is_hi.sum(axis=1)                            # [npad]

    def wrap(arr):
        # arr [n] -> [128, n//16] int16 (16-wrap replicated to 128)
        n = arr.shape[0]
        w16 = arr.reshape(n // 16, 16).T.astype(np.int16)
        return np.tile(w16, (8, 1))

    in_maps = []
    metas = []
    for cidx in range(n_cores):
        sl = slice(cidx * per_core, (cidx + 1) * per_core)
        # sort this core's points by hi-count so tiles are homogeneous
        hc = hi_cnt[sl]
        perm = np.argsort(hc, kind="stable")              # local indices
        gperm = cidx * per_core + perm                    # global ids
        nis = ni_sorted[gperm]                            # [per_core, K]

        hi_n = []
        lo_cols = np.zeros((128, n_tiles * 128), np.int16)
        ent_core = np.zeros((n_tiles, NPAIR), np.int64)
        ru_blocks = np.zeros((2, n_tiles * 2 * NPAIR), NPF8)
        ruf_blocks = np.zeros((4, n_tiles * NPAIR), np.float16)
        for t in range(n_tiles):
            pts = nis[t * PT_TILE:(t + 1) * PT_TILE]      # [128, K]
            # k-major pair order: col j = 128*k + pt -> entry pts[pt, k]
            ent_t = pts.T.reshape(-1)                     # [2048] (k-major)
            ent_core[t] = ent_t
            lo_v = np.where(ent_t < THRESH, ent_t + 1, 0).astype(np.int16)
            lo_cols[:, t * 128:(t + 1) * 128] = wrap(lo_v)
            mh = int(hi_cnt[gperm[t * PT_TILE:(t + 1) * PT_TILE]].max())
            hi_n.append(128 * mh)
            # ru for this tile, k-major
            ctr = np.repeat(gperm[t * PT_TILE:(t + 1) * PT_TILE][None, :],
                            K, axis=0).reshape(-1)        # [2048]
            u = a[ent_t] - actr[ctr]                      # [2048, 3]
            ru = np.maximum(u, 0.0).astype(np.float32)
            ru4 = np.concatenate([ru, np.ones((NPAIR, 1), np.float32)],
                                 axis=1)                  # [2048, 4]
            blk = ru4.reshape(NPAIR, 2, 2).transpose(1, 0, 2)  # [2, 2048, 2]
            ru_blocks[:, t * 2 * NPAIR:(t + 1) * 2 * NPAIR] = \
                blk.reshape(2, 2 * NPAIR).astype(NPF8)
            ruf_blocks[:, t * NPAIR:(t + 1) * NPAIR] = ru4.T.astype(np.float16)

        ftw = np.ascontiguousarray(feats0[gperm].T.astype(np.float16))

        in_maps.append({
            "table_lo": table_lo, "table_hi": table_hi,
            "idx_lo": lo_cols,
            "ru4": ru_blocks, "ruf": ruf_blocks, "ftw": ftw,
            "wk_dr": f["wk_dr"], "wv_f16": f["wv_f16"],
            "wq_f16": f["wq_f16"],
            "wp2w_dr": f["wp2w_dr"], "wp2v_f16": f["wp2v_f16"],
            "ww1z": f["ww1z"], "w2r": f["w2r"],
            "b_w": f["b_w"].reshape(C, 1),
            "b1f_pk": f["b1f_pk"].reshape(C, 1),
            "be_rep": f["be_rep"].reshape(C, 1),
        })
        metas.append(dict(hi_n=tuple(hi_n), perm=perm, ent=ent_core,
                          wrap=wrap))

    meta = dict(n_tiles=n_tiles, per_core=per_core, npad=npad,
                lo_rows=lo_rows, hi_rows=hi_rows, n_real=n_real,
                hi_ns=tuple(m["hi_n"] for m in metas),
                perms=[m["perm"] for m in metas],
                ents=[m["ent"] for m in metas], wrap=wrap)
    return in_maps, meta


# ------------------------------------------------------------- walrus compat
def split_excess_waits(nc, max_waits=1):
    n_split = 0
    for fn in nc.m.functions:
        for blk in fn.blocks:
            new_insts = []
            for inst in blk.instructions:
                si = inst.sync_info
                lim = (1 if isinstance(inst, (mybir.InstDrain, mybir.InstNoOp,
                                              mybir.InstEventSemaphore))
                       else max_waits)
                if si is not None and si.on_wait and len(si.on_wait) > lim:
                    waits = list(si.on_wait)
                    extra, keep = waits[:-lim], waits[-lim:]
                    ci = 0
                    while extra:
                        chunk, extra = extra[:1], extra[1:]
                        new_insts.append(mybir.InstNoOp(
                            name=f"{inst.name}-waitsplit{ci}",
                            engine=inst.engine,
                            bass_nofuse=True,
                            sync_info=mybir.SyncInfo(on_wait=chunk, on_update=[]),
                        ))
                        ci += 1
                    si.on_wait = keep
                    n_split += 1
                new_insts.append(inst)
            blk.instructions = new_insts
    return n_split


# ----------------------------------------------------------------- the kernel
def build_nc(meta, hi_n):
    from concourse import library_config
    n_tiles = meta["n_tiles"]
    per_core = meta["per_core"]
    hi_tot = sum(hi_n)
    hi_off = np.concatenate([[0], np.cumsum(hi_n)]).astype(int)
    nc = bass.Bass("TRN2", target_bir_lowering=False, debug=False,
                   num_swdge_queues=1)

    dt_ = nc.dram_tensor
    t_lo = dt_("table_lo", [meta["lo_rows"], 256], F16, kind="ExternalInput").ap()
    t_hi = dt_("table_hi", [meta["hi_rows"], 256], F16, kind="ExternalInput").ap()
    idx_lo = dt_("idx_lo", [128, n_tiles * 128], I16, kind="ExternalInput").ap()
    idx_hi_cols = max(hi_tot // 16, 8)
    idx_hi = dt_("idx_hi", [128, idx_hi_cols], I16, kind="ExternalInput").ap()
    ru4_d = dt_("ru4", [2, n_tiles * 2 * NPAIR], FP8, kind="ExternalInput").ap()
    ruf_d = dt_("ruf", [4, n_tiles * NPAIR], F16, kind="ExternalInput").ap()
    ftw_d = dt_("ftw", [C, per_core], F16, kind="ExternalInput").ap()
    wk_d = dt_("wk_dr", [64, 2, C], FP8, kind="ExternalInput").ap()
    wv_d = dt_("wv_f16", [C, C], F16, kind="ExternalInput").ap()
    wq_d = dt_("wq_f16", [C, C], F16, kind="ExternalInput").ap()
    wp2w_d = dt_("wp2w_dr", [2, 2, C], FP8, kind="ExternalInput").ap()
    wp2v_d = dt_("wp2v_f16", [4, C], F16, kind="ExternalInput").ap()
    ww1z_d = dt_("ww1z", [C, 32], F16, kind="ExternalInput").ap()
    w2r_d = dt_("w2r", [C, 4 * C], F16, kind="ExternalInput").ap()
    b_w_d = dt_("b_w", [C, 1], F32, kind="ExternalInput").ap()
    b1f_d = dt_("b1f_pk", [C, 1], F32, kind="ExternalInput").ap()
    be_d = dt_("be_rep", [C, 1], F32, kind="ExternalInput").ap()
    outT = dt_("outT", [C, per_core], F32, kind="ExternalOutput").ap()

    Relu = mybir.ActivationFunctionType.Relu
    Exp = mybir.ActivationFunctionType.Exp
    DR = mybir.MatmulPerfMode.DoubleRow
    ADD = mybir.AluOpType.add
    MULT = mybir.AluOpType.mult
    MAX = mybir.AluOpType.max

    nc.gpsimd.load_library(library_config.mlp)
    rlo = nc.gpsimd.alloc_register("nlo")
    nc.gpsimd.reg_mov(rlo, NPAIR)
    rhi = nc.gpsimd.alloc_register("nhi")

    with tile.TileContext(nc) as tc:
        with (
            tc.tile_pool(name="const", bufs=1) as cpool,
            tc.tile_pool(name="gath", bufs=2) as gpool,
            tc.tile_pool(name="ru", bufs=2) as rupool,
            tc.tile_pool(name="work", bufs=2) as wpool,
            tc.tile_pool(name="tail", bufs=2) as tpool,
            tc.tile_pool(name="psW", bufs=1, space="PSUM") as psW,
            tc.tile_pool(name="psV", bufs=2, space="PSUM") as psV,
            tc.tile_pool(name="psX", bufs=1, space="PSUM") as psX,
        ):
            def cload(ap_dram, shape, dtype, tag):
                t = cpool.tile(shape, dtype, tag=tag)
                nc.sync.dma_start(t[:], ap_dram)
                return t

            wk = cload(wk_d, [64, 2, C], FP8, "wk")
            wv = cload(wv_d, [C, C], F16, "wv")
            wq = cload(wq_d, [C, C], F16, "wq")
            wp2w = cload(wp2w_d, [2, 2, C], FP8, "wp2w")
            wp2v = cload(wp2v_d, [4, C], F16, "wp2v")
            ww1z = cload(ww1z_d, [C, 32], F16, "ww1z")
            w2r = cload(w2r_d, [C, 4 * C], F16, "w2r")
            b_w = cload(b_w_d, [C, 1], F32, "b_w")
            b1f = cload(b1f_d, [C, 1], F32, "b1f")
            be_b = cload(be_d, [C, 1], F32, "be_b")
            ixlo = cload(idx_lo, [128, n_tiles * 128], I16, "ixlo")
            ixhi = cload(idx_hi, [128, idx_hi_cols], I16, "ixhi")
            ftw = cpool.tile([C, per_core], F16, tag="ftw")
            nc.sync.dma_start(ftw[:], ftw_d)

            state = {}

            def s0_gather(t):
                glo = gpool.tile([128, 2, NPAIR], F16, tag="glo")
                nc.gpsimd.dma_gather(glo[:], t_lo,
                                     ixlo[:, bass.ts(t, 128)], NPAIR, rlo,
                                     256, transpose=True, queue_num=0,
                                     single_packet=False)
                hn = hi_n[t]
                ghi = None
                if hn > 0:
                    ghi = gpool.tile([128, 2, hn], F16, tag="ghi")
                    nc.gpsimd.reg_mov(rhi, hn)
                    nc.gpsimd.dma_gather(
                        ghi[:], t_hi,
                        ixhi[:, hi_off[t] // 16:hi_off[t + 1] // 16], hn, rhi,
                        256, transpose=True, queue_num=0,
                        single_packet=False)
                ru4 = rupool.tile([2, 2 * NPAIR], FP8, tag="ru4")
                nc.sync.dma_start(ru4[:],
                                  ru4_d[:, t * 2 * NPAIR:(t + 1) * 2 * NPAIR])
                ruf = rupool.tile([4, NPAIR], F16, tag="ruf")
                nc.sync.dma_start(ruf[:], ruf_d[:, t * NPAIR:(t + 1) * NPAIR])
                state[("g", t)] = (glo, ghi, ru4, ruf)

            def s1_tile(t):
                glo, ghi, ru4, ruf = state.pop(("g", t))
                hn = hi_n[t]
                # k-path fp8 channel-pair views + v-path f16 views
                glo8 = glo[0:64, 0, :].bitcast(FP8)   # [64, 2*NPAIR]
                glo16 = glo[:, 1, :]                  # [128, NPAIR] f16
                ghi8 = ghi[0:64, 0, :].bitcast(FP8) if hn else None
                ghi16 = ghi[:, 1, :] if hn else None

                SP = []

                for ch in range(NPAIR // CHUNK):
                    c0 = ch * CHUNK
                    # --- v path first: vps PSUM is double-buffered so the
                    # next chunk's matmuls never wait on this chunk's t2
                    vps = psV.tile([C, CHUNK], F32, tag="vps")
                    for hf in range(CHUNK // MM):
                        lo = c0 + hf * MM
                        o = vps[:, hf * MM:(hf + 1) * MM]
                        nc.tensor.matmul(o, wv[:], glo16[:, lo:lo + MM],
                                         start=True, stop=False)
                        nc.tensor.matmul(
                            o, wp2v[:], ruf[:, lo:lo + MM],
                            start=False, stop=(hn == 0 or lo + MM <= NPAIR - hn))
                        if hn and lo + MM > NPAIR - hn:
                            ho = max(NPAIR - hn, lo)
                            hb = NPAIR - hn
                            nc.tensor.matmul(
                                vps[:, ho - c0:(hf + 1) * MM], wv[:],
                                ghi16[:, ho - hb:lo + MM - hb],
                                start=False, stop=True)
                    # --- w path: wps = scaled (kf - q + p) accumulation
                    wps = psW.tile([C, CHUNK], F32, tag="wps")
                    for hf in range(CHUNK // MM):
                        lo = c0 + hf * MM
                        o = wps[:, hf * MM:(hf + 1) * MM]
                        nc.tensor.matmul(
                            o, wk[:],
                            glo8[:, 2 * lo:2 * (lo + MM)]
                            .rearrange("p (n two) -> p two n", two=2),
                            start=True, stop=False, perf_mode=DR)
                        # q broadcast: col 128k+pt -> ftw col pt
                        qv = (ftw[:, t * PT_TILE:(t + 1) * PT_TILE]
                              .unsqueeze(1).broadcast_to([C, MM // PT_TILE,
                                                          PT_TILE]))
                        nc.tensor.matmul(o, wq[:], qv, start=False, stop=False)
                        nc.tensor.matmul(
                            o, wp2w[:],
                            ru4[:, 2 * lo:2 * (lo + MM)]
                            .rearrange("p (n two) -> p two n", two=2),
                            start=False, stop=(hn == 0 or lo + MM <= NPAIR - hn),
                            perf_mode=DR)
                        if hn and lo + MM > NPAIR - hn:
                            ho = max(NPAIR - hn, lo)
                            hb = NPAIR - hn
                            nc.tensor.matmul(
                                wps[:, ho - c0:(hf + 1) * MM], wk[:],
                                glo8x := ghi8[:, 2 * (ho - hb):2 * (lo + MM - hb)]
                                .rearrange("p (n two) -> p two n", two=2),
                                start=False, stop=True, perf_mode=DR)
                    r_t = wpool.tile([C, CHUNK], F16, tag="r")
                    nc.scalar.activation(r_t[:], wps[:], Relu,
                                         bias=b_w[:], scale=1.0 / SCALE)
                    # --- h packed [128, 256]: 4 groups of 256 cols at
                    # 32-aligned partition offsets (16 real + 16 zero rows)
                    hps = psX.tile([C, 2 * PT_TILE], F32, tag="scr")
                    for g in range(4):
                        nc.tensor.matmul(
                            hps[32 * g:32 * g + 32, :], ww1z[:],
                            r_t[:, 2 * g * PT_TILE:2 * (g + 1) * PT_TILE],
                            start=True, stop=True, tile_position=(0, 32 * g))
                    h2 = wpool.tile([C, 2 * PT_TILE], F16, tag="h2")
                    nc.scalar.activation(h2[:], hps[:], Relu, bias=b1f[:])
                    # --- replicated logits [128, CHUNK] from packed h
                    lps = psX.tile([C, CHUNK], F32, tag="scr")
                    for g in range(4):
                        nc.tensor.matmul(
                            lps[:, 2 * g * PT_TILE:2 * (g + 1) * PT_TILE],
                            w2r[:, g * C:(g + 1) * C], h2[:],
                            start=True, stop=True)
                    # e and t2 share one tile so a single tree reduces both
                    et = wpool.tile([C, 2, CHUNK], F16, tag="et")
                    e_sb = et[:, 0, :]
                    nc.scalar.activation(e_sb, lps[:], Exp, bias=be_b[:])
                    nc.vector.tensor_tensor(et[:, 1, :], e_sb, vps[:], MULT)
                    # --- fused K-tree over [e | t2]: [C,2,8,128]->[C,2,128]
                    ev = et[:].rearrange("p u (a b) -> p u a b", b=PT_TILE)
                    f4 = wpool.tile([C, 2, 4 * PT_TILE], F16, tag="f4")
                    f4v = f4[:].rearrange("p u (a b) -> p u a b", b=PT_TILE)
                    nc.vector.tensor_tensor(f4v, ev[:, :, 0:4], ev[:, :, 4:8],
                                            ADD)
                    f8 = wpool.tile([C, 2, 2 * PT_TILE], F16, tag="f8")
                    f8v = f8[:].rearrange("p u (a b) -> p u a b", b=PT_TILE)
                    nc.vector.tensor_tensor(f8v, f4v[:, :, 0:2], f4v[:, :, 2:4],
                                            ADD)
                    sa = tpool.tile([C, 2, PT_TILE], F16, tag=f"SA{ch}")
                    sav = sa[:]
                    nc.vector.tensor_tensor(sav, f8v[:, :, 0], f8v[:, :, 1],
                                            ADD)
                    SP.append(sa)

                # ---- tail
                su = tpool.tile([C, 2, PT_TILE], F16, tag="su")
                nc.vector.tensor_tensor(su[:], SP[0][:], SP[1][:], ADD)
                rS = tpool.tile([C, PT_TILE], F16, tag="rS")
                with nc.allow_low_precision("softmax denom recip in f16"):
                    nc.vector.reciprocal(rS[:], su[:, 0, :])
                # aggN = aggU * rS / SCALE  (rS is 1/(S_true/SCALE))
                aggN = tpool.tile([C, PT_TILE], F16, tag="aggN")
                nc.vector.scalar_tensor_tensor(aggN[:], su[:, 1, :],
                                               1.0 / SCALE, rS[:], MULT, MULT)
                l2 = tpool.tile([C, PT_TILE], F16, tag="l2")
                nc.vector.tensor_tensor(l2[:], aggN[:],
                                        ftw[:, bass.ts(t, PT_TILE)], ADD)
                outc = tpool.tile([C, PT_TILE], F32, tag="outc")
                nc.vector.scalar_tensor_tensor(outc[:], l2[:], 0.1, l2[:],
                                               MULT, MAX)
                nc.sync.dma_start(outT[:, bass.ts(t, PT_TILE)], outc[:])

            for i in range(n_tiles + 1):
                if i < n_tiles:
                    s0_gather(i)
                if i >= 1:
                    s1_tile(i - 1)

    from concourse.library_overlay import lower_extended_insts
    lower_extended_insts(nc)
    split_excess_waits(nc)
    return nc


# ------------------------------------------------------------- entry point
_CACHE = {}


def kernel(**inputs) -> np.ndarray:
    from concourse.bass_utils import run_bass_kernel_spmd

    xyz = np.asarray(inputs["xyz"], np.float32)
    feats = np.asarray(inputs["feats"], np.float32)
    nei = np.asarray(inputs["nei_ind"])
    params = {k: np.asarray(v, np.float32) for k, v in inputs.items()
              if k not in ("xyz", "feats", "nei_ind")}

    in_maps, meta = prep_inputs(xyz, feats, nei, params, N_CORES)

    # one compiled program per distinct hi_n profile; all cores share the
    # max profile so a single SPMD binary serves all 8
    hi_max = tuple(int(max(meta["hi_ns"][c][t] for c in range(N_CORES)))
                   for t in range(meta["n_tiles"]))
    key = (meta["n_tiles"], meta["per_core"], hi_max)
    if key not in _CACHE:
        _CACHE[key] = build_nc(meta, hi_max)
    nc = _CACHE[key]

    # build each core's hi idx stream against the shared profile
    hi_off_shared = np.concatenate([[0], np.cumsum(hi_max)]).astype(int)
    tot = max(int(hi_off_shared[-1]) // 16, 8)
    for cidx in range(N_CORES):
        dst = np.zeros((128, tot), np.int16)
        ent_core = meta["ents"][cidx]
        for t in range(meta["n_tiles"]):
            hn = hi_max[t]
            if hn:
                hv = ent_core[t][NPAIR - hn:]
                hv = np.where(hv >= THRESH, hv - THRESH + 1, 0
                              ).astype(np.int16)
                dst[:, hi_off_shared[t] // 16:hi_off_shared[t + 1] // 16] = \
                    meta["wrap"](hv)
        in_maps[cidx]["idx_hi"] = dst

    res = run_bass_kernel_spmd(nc, in_maps, core_ids=list(range(N_CORES)))
    outs = []
    for cidx, r in enumerate(res.results):
        o = np.asarray(r["outT"]).T                      # [per_core, C]
        inv = np.empty_like(meta["perms"][cidx])
        inv[meta["perms"][cidx]] = np.arange(len(inv))
        outs.append(o[inv])
    full = np.concatenate(outs, axis=0)                  # [npad, C]
    return np.ascontiguousarray(full[None, :meta["n_real"]]).astype(np.float32)


# revision 41
# speedup vs baseline: 1.4442x; 1.0908x over previous
"""PointTransformerLayer Bass kernel for TRN2 (v2).

Per-core design (points sharded 8 ways, table replicated):
  - DRAM gather table: one 256B row per point = 128 fp8(e4m3) feats packed
    as channel pairs (u16 word w = channels 2w, 2w+1).  Gathered in
    transpose mode the tile is natively in fp8-DoubleRow moving-operand
    layout, so Wk/Wv matmuls run at 0.5 cyc/col.
  - int16 index reach is 32767 rows, so the table is split lo/hi with a
    zero row at index 0.  Per point the 16 neighbor slots are reordered
    (softmax over K is permutation invariant) so hi-table refs occupy the
    top k-blocks; points are sorted per core by hi-count so each 128-pt
    tile needs only hi_n[t] = 128*max_hi descriptors for the hi gather.
    No on-chip merge: lo and hi gathered tiles are separately accumulated
    into PSUM by the (linear) Wk/Wv matmuls; missing slots fetch zero rows.
  - Pair order within a tile is k-major: col = 128*k + pt.
  - The position term relu(a_nbr - a_ctr) is precomputed on host and
    streamed as a [2,2,N] fp8 DoubleRow operand (3 channels + ones; the
    ones row carries nothing for w (bias via ACT) and 16*bvp for v).
  - All fp8 weights carry a x16 scale; undone via ACT scale (w path),
    exp bias (e path) and reciprocal scale (normalization).
  - Back-end packing: h = relu(Ww1'r) lands as [128,128] PSUM (8 matmuls
    with partition offsets), Ww2 is applied block-diagonally (128 cols),
    exp runs on [128,128], softmax denom comes from a one-hot reduction
    matmul accumulated across chunks, and e is re-broadcast to [128,1024]
    PSUM by 8 one-hot matmuls.
  - t2 = e*v on DVE straight from PSUM; K-reduction = 3-level strided
    tree adds in f16.
"""

import sys

sys.path.insert(0, "/opt/trn_rl_repo")
sys.path.insert(0, "/root/.axon_site/_ro/trn_rl_repo")

import numpy as np
import ml_dtypes

import concourse.bass as bass
import concourse.tile as tile
from concourse import mybir

F16 = mybir.dt.float16
F32 = mybir.dt.float32
FP8 = mybir.dt.float8e4
I16 = mybir.dt.int16
NPF8 = ml_dtypes.float8_e4m3

K = 16
C = 128
S = 8
CS = C // S  # 16
EPS = 1e-5
EXP_SHIFT = float(np.log(256.0))
SCALE = 16.0
LN_SCALE = float(np.log(SCALE))
PT_TILE = 128
NPAIR = PT_TILE * K     # 2048
CHUNK = 1024
MM = 512
NBLK = NPAIR // PT_TILE  # 16 k-blocks per tile
THRESH = 32767           # entries in lo table (idx = e+1 <= 32767)

N_CORES = 8


# ----------------------------------------------------------------- host math
def fold_params(p):
    f32 = np.float32
    s_p = (p["p_gamma"] / np.sqrt(p["p_var"] + EPS)).astype(f32)
    Afold = (p["Wp1"] * s_p[None, :]).astype(f32)
    cfold = ((p["bp1"] - p["p_mean"]) * s_p + p["p_beta"]).astype(f32)

    s_w = (p["w_gamma"] / np.sqrt(p["w_var"] + EPS)).astype(f32)
    ball = (p["bk"] - p["bq"] + p["bp2"]).astype(f32)
    b_w = ((ball - p["w_mean"]) * s_w + p["w_beta"]).astype(f32)

    s1 = (p["w1_gamma"] / np.sqrt(p["w1_var"] + EPS)).astype(f32)
    ww1s = (p["Ww1"] * s1[None, :]).astype(np.float16)        # [128, 16]
    b1f = ((p["bw1"] - p["w1_mean"]) * s1 + p["w1_beta"]).astype(f32)

    bvp = (p["bv"] + p["bp2"]).astype(f32)                    # [128]

    def pack_dr(w):  # [in_ch, out] f32 -> [in_ch//2, 2, out] fp8
        ic = w.shape[0]
        return np.ascontiguousarray(
            w.reshape(ic // 2, 2, w.shape[1])).astype(NPF8)

    wk_dr = pack_dr(SCALE * s_w[None, :] * p["Wk"])           # [64,2,128]
    wv_f16 = (SCALE * p["Wv"]).astype(np.float16)             # [128,128]
    wq_f16 = (-SCALE * s_w[None, :] * p["Wq"]).astype(np.float16)  # [128,128]

    # ru4 channels: 0..2 = relu(u), 3 = ones
    wp2w = np.zeros((4, C), np.float32)
    wp2w[:3] = SCALE * s_w[None, :] * p["Wp2"]
    wp2w_dr = pack_dr(wp2w)                                   # [2,2,128]
    wp2v = np.zeros((4, C), np.float32)
    wp2v[:3] = SCALE * p["Wp2"]
    wp2v[3] = SCALE * bvp
    wp2v_f16 = wp2v.astype(np.float16)                        # [4,128]

    # packed h layout: 4 groups of 32 partitions (16 real + 16 zero hole);
    # group g covers pair-columns [256g, 256g+256) of a 1024-col chunk.
    ww1z = np.zeros((C, 32), np.float16)       # Ww1 + 16 zero out-cols
    ww1z[:, :16] = ww1s
    b1f_pk = np.zeros(C, f32)
    for g in range(4):
        b1f_pk[32 * g:32 * g + 16] = b1f
    # replicated-logits stationaries: block g maps packed h (partitions
    # 32g+i) to all 128 replicated logit channels for its 256 columns
    w2r = np.zeros((C, 4 * C), np.float16)
    for g in range(4):
        for c in range(C):
            for i in range(16):
                w2r[32 * g + i, g * C + c] = p["Ww2"][i, c % 16]
    be_rep = (np.array([p["bw2"][c % 16] for c in range(C)], f32)
              - EXP_SHIFT - LN_SCALE)

    return dict(
        Afold=Afold, cfold=cfold, b_w=b_w, ww1z=ww1z,
        wk_dr=wk_dr, wv_f16=wv_f16, wq_f16=wq_f16,
        wp2w_dr=wp2w_dr, wp2v_f16=wp2v_f16,
        be_rep=be_rep, b1f_pk=b1f_pk, w2r=w2r,
    )


def prep_inputs(xyz, feats, nei_ind, params, n_cores):
    f = fold_params(params)
    n_real = feats.shape[1]
    per_core_raw = -(-n_real // n_cores)
    per_core = -(-per_core_raw // PT_TILE) * PT_TILE
    npad = per_core * n_cores
    n_tiles = per_core // PT_TILE

    feats0 = np.zeros((npad, C), np.float32)
    feats0[:n_real] = feats[0]
    pos0 = np.zeros((npad, 3), np.float32)
    pos0[:n_real] = xyz[0]
    ni = np.zeros((npad, K), np.int64)
    ni[:n_real] = nei_ind[0]

    a = (pos0 @ f["Afold"]).astype(np.float32)            # [npad, 3]
    actr = (a - f["cfold"][None, :]).astype(np.float32)   # center role

    # table rows (512B): words 0..63 = fp8 channel-paired feats (k path),
    # words 128..255 = f16 feats (v path)
    feats8 = feats0.astype(NPF8)                          # [npad, 128] fp8
    ent = np.zeros((npad, 256), np.uint16)
    ent[:, :64] = feats8.view(np.uint8)[:, 0::2].astype(np.uint16) \
        | (feats8.view(np.uint8)[:, 1::2].astype(np.uint16) << 8)
    ent[:, 128:256] = feats0.astype(np.float16).view(np.uint16)
    lo_rows = THRESH + 1
    hi_rows = npad - THRESH + 1
    table_lo = np.zeros((lo_rows, 256), np.uint16)
    table_lo[1:] = ent[:THRESH]
    table_hi = np.zeros((hi_rows, 256), np.uint16)
    table_hi[1:] = ent[THRESH:]
    table_lo = table_lo.view(np.float16)
    table_hi = table_hi.view(np.float16)

    # per-point slot sort: lo refs first, hi refs last
    is_hi = ni >= THRESH                                  # [npad, K]
    slot_order = np.argsort(is_hi, axis=1, kind="stable") # [npad, K]
    ni_sorted = np.take_along_axis(ni, slot_order, axis=1)
    hi_cnt = is_hi.sum(axis=1)                            # [npad]

    def wrap(arr):
        # arr [n] -> [128, n//16] int16 (16-wrap replicated to 128)
        n = arr.shape[0]
        w16 = arr.reshape(n // 16, 16).T.astype(np.int16)
        return np.tile(w16, (8, 1))

    in_maps = []
    metas = []
    for cidx in range(n_cores):
        sl = slice(cidx * per_core, (cidx + 1) * per_core)
        # sort this core's points by hi-count so tiles are homogeneous
        hc = hi_cnt[sl]
        perm = np.argsort(hc, kind="stable")              # local indices
        gperm = cidx * per_core + perm                    # global ids
        nis = ni_sorted[gperm]                            # [per_core, K]

        hi_n = []
        lo_cols = np.zeros((128, n_tiles * 128), np.int16)
        ent_core = np.zeros((n_tiles, NPAIR), np.int64)
        ru_blocks = np.zeros((2, n_tiles * 2 * NPAIR), NPF8)
        ruf_blocks = np.zeros((4, n_tiles * NPAIR), np.float16)
        for t in range(n_tiles):
            pts = nis[t * PT_TILE:(t + 1) * PT_TILE]      # [128, K]
            # k-major pair order: col j = 128*k + pt -> entry pts[pt, k]
            ent_t = pts.T.reshape(-1)                     # [2048] (k-major)
            ent_core[t] = ent_t
            lo_v = np.where(ent_t < THRESH, ent_t + 1, 0).astype(np.int16)
            lo_cols[:, t * 128:(t + 1) * 128] = wrap(lo_v)
            mh = int(hi_cnt[gperm[t * PT_TILE:(t + 1) * PT_TILE]].max())
            hi_n.append(128 * mh)
            # ru for this tile, k-major
            ctr = np.repeat(gperm[t * PT_TILE:(t + 1) * PT_TILE][None, :],
                            K, axis=0).reshape(-1)        # [2048]
            u = a[ent_t] - actr[ctr]                      # [2048, 3]
            ru = np.maximum(u, 0.0).astype(np.float32)
            ru4 = np.concatenate([ru, np.ones((NPAIR, 1), np.float32)],
                                 axis=1)                  # [2048, 4]
            blk = ru4.reshape(NPAIR, 2, 2).transpose(1, 0, 2)  # [2, 2048, 2]
            ru_blocks[:, t * 2 * NPAIR:(t + 1) * 2 * NPAIR] = \
                blk.reshape(2, 2 * NPAIR).astype(NPF8)
            ruf_blocks[:, t * NPAIR:(t + 1) * NPAIR] = ru4.T.astype(np.float16)

        ftw = np.ascontiguousarray(feats0[gperm].T.astype(np.float16))

        in_maps.append({
            "table_lo": table_lo, "table_hi": table_hi,
            "idx_lo": lo_cols,
            "ru4": ru_blocks, "ruf": ruf_blocks, "ftw": ftw,
            "wk_dr": f["wk_dr"], "wv_f16": f["wv_f16"],
            "wq_f16": f["wq_f16"],
            "wp2w_dr": f["wp2w_dr"], "wp2v_f16": f["wp2v_f16"],
            "ww1z": f["ww1z"], "w2r": f["w2r"],
            "b_w": f["b_w"].reshape(C, 1),
            "b1f_pk": f["b1f_pk"].reshape(C, 1),
            "be_rep": f["be_rep"].reshape(C, 1),
        })
        metas.append(dict(hi_n=tuple(hi_n), perm=perm, ent=ent_core,
                          wrap=wrap))

    meta = dict(n_tiles=n_tiles, per_core=per_core, npad=npad,
                lo_rows=lo_rows, hi_rows=hi_rows, n_real=n_real,
                hi_ns=tuple(m["hi_n"] for m in metas),
                perms=[m["perm"] for m in metas],
                ents=[m["ent"] for m in metas], wrap=wrap)
    return in_maps, meta


# ------------------------------------------------------------- walrus compat
def split_excess_waits(nc, max_waits=1):
    n_split = 0
    for fn in nc.m.functions:
        for blk in fn.blocks:
            new_insts = []
            for inst in blk.instructions:
                si = inst.sync_info
                lim = (1 if isinstance(inst, (mybir.InstDrain, mybir.InstNoOp,
                                              mybir.InstEventSemaphore))
                       else max_waits)
                if si is not None and si.on_wait and len(si.on_wait) > lim:
                    waits = list(si.on_wait)
                    extra, keep = waits[:-lim], waits[-lim:]
                    ci = 0
                    while extra:
                        chunk, extra = extra[:1], extra[1:]
                        new_insts.append(mybir.InstNoOp(
                            name=f"{inst.name}-waitsplit{ci}",
                            engine=inst.engine,
                            bass_nofuse=True,
                            sync_info=mybir.SyncInfo(on_wait=chunk, on_update=[]),
                        ))
                        ci += 1
                    si.on_wait = keep
                    n_split += 1
                new_insts.append(inst)
            blk.instructions = new_insts
    return n_split


# ----------------------------------------------------------------- the kernel
def build_nc(meta, hi_n):
    from concourse import library_config
    n_tiles = meta["n_tiles"]
    per_core = meta["per_core"]
    hi_tot = sum(hi_n)
    hi_off = np.concatenate([[0], np.cumsum(hi_n)]).astype(int)
    nc = bass.Bass("TRN2", target_bir_lowering=False, debug=False,
                   num_swdge_queues=1)

    dt_ = nc.dram_tensor
    t_lo = dt_("table_lo", [meta["lo_rows"], 256], F16, kind="ExternalInput").ap()
    t_hi = dt_("table_hi", [meta["hi_rows"], 256], F16, kind="ExternalInput").ap()
    idx_lo = dt_("idx_lo", [128, n_tiles * 128], I16, kind="ExternalInput").ap()
    idx_hi_cols = max(hi_tot // 16, 8)
    idx_hi = dt_("idx_hi", [128, idx_hi_cols], I16, kind="ExternalInput").ap()
    ru4_d = dt_("ru4", [2, n_tiles * 2 * NPAIR], FP8, kind="ExternalInput").ap()
    ruf_d = dt_("ruf", [4, n_tiles * NPAIR], F16, kind="ExternalInput").ap()
    ftw_d = dt_("ftw", [C, per_core], F16, kind="ExternalInput").ap()
    wk_d = dt_("wk_dr", [64, 2, C], FP8, kind="ExternalInput").ap()
    wv_d = dt_("wv_f16", [C, C], F16, kind="ExternalInput").ap()
    wq_d = dt_("wq_f16", [C, C], F16, kind="ExternalInput").ap()
    wp2w_d = dt_("wp2w_dr", [2, 2, C], FP8, kind="ExternalInput").ap()
    wp2v_d = dt_("wp2v_f16", [4, C], F16, kind="ExternalInput").ap()
    ww1z_d = dt_("ww1z", [C, 32], F16, kind="ExternalInput").ap()
    w2r_d = dt_("w2r", [C, 4 * C], F16, kind="ExternalInput").ap()
    b_w_d = dt_("b_w", [C, 1], F32, kind="ExternalInput").ap()
    b1f_d = dt_("b1f_pk", [C, 1], F32, kind="ExternalInput").ap()
    be_d = dt_("be_rep", [C, 1], F32, kind="ExternalInput").ap()
    outT = dt_("outT", [C, per_core], F32, kind="ExternalOutput").ap()

    Relu = mybir.ActivationFunctionType.Relu
    Exp = mybir.ActivationFunctionType.Exp
    DR = mybir.MatmulPerfMode.DoubleRow
    ADD = mybir.AluOpType.add
    MULT = mybir.AluOpType.mult
    MAX = mybir.AluOpType.max

    nc.gpsimd.load_library(library_config.mlp)
    rlo = nc.gpsimd.alloc_register("nlo")
    nc.gpsimd.reg_mov(rlo, NPAIR)
    rhi = nc.gpsimd.alloc_register("nhi")

    with tile.TileContext(nc) as tc:
        with (
            tc.tile_pool(name="const", bufs=1) as cpool,
            tc.tile_pool(name="gath", bufs=3) as gpool,
            tc.tile_pool(name="ru", bufs=3) as rupool,
            tc.tile_pool(name="work", bufs=2) as wpool,
            tc.tile_pool(name="tail", bufs=2) as tpool,
            tc.tile_pool(name="psW", bufs=1, space="PSUM") as psW,
            tc.tile_pool(name="psV", bufs=2, space="PSUM") as psV,
            tc.tile_pool(name="psX", bufs=1, space="PSUM") as psX,
        ):
            def cload(ap_dram, shape, dtype, tag):
                t = cpool.tile(shape, dtype, tag=tag)
                nc.sync.dma_start(t[:], ap_dram)
                return t

            wk = cload(wk_d, [64, 2, C], FP8, "wk")
            wv = cload(wv_d, [C, C], F16, "wv")
            wq = cload(wq_d, [C, C], F16, "wq")
            wp2w = cload(wp2w_d, [2, 2, C], FP8, "wp2w")
            wp2v = cload(wp2v_d, [4, C], F16, "wp2v")
            ww1z = cload(ww1z_d, [C, 32], F16, "ww1z")
            w2r = cload(w2r_d, [C, 4 * C], F16, "w2r")
            b_w = cload(b_w_d, [C, 1], F32, "b_w")
            b1f = cload(b1f_d, [C, 1], F32, "b1f")
            be_b = cload(be_d, [C, 1], F32, "be_b")
            ixlo = cload(idx_lo, [128, n_tiles * 128], I16, "ixlo")
            ixhi = cload(idx_hi, [128, idx_hi_cols], I16, "ixhi")
            ftw = cpool.tile([C, per_core], F16, tag="ftw")
            nc.sync.dma_start(ftw[:], ftw_d)

            state = {}

            def s0_gather(t):
                glo = gpool.tile([128, 2, NPAIR], F16, tag="glo")
                nc.gpsimd.dma_gather(glo[:], t_lo,
                                     ixlo[:, bass.ts(t, 128)], NPAIR, rlo,
                                     256, transpose=True, queue_num=0,
                                     single_packet=False)
                hn = hi_n[t]
                ghi = None
                if hn > 0:
                    ghi = gpool.tile([128, 2, hn], F16, tag="ghi")
                    nc.gpsimd.reg_mov(rhi, hn)
                    nc.gpsimd.dma_gather(
                        ghi[:], t_hi,
                        ixhi[:, hi_off[t] // 16:hi_off[t + 1] // 16], hn, rhi,
                        256, transpose=True, queue_num=0,
                        single_packet=False)
                ru4 = rupool.tile([2, 2 * NPAIR], FP8, tag="ru4")
                nc.sync.dma_start(ru4[:],
                                  ru4_d[:, t * 2 * NPAIR:(t + 1) * 2 * NPAIR])
                ruf = rupool.tile([4, NPAIR], F16, tag="ruf")
                nc.sync.dma_start(ruf[:], ruf_d[:, t * NPAIR:(t + 1) * NPAIR])
                state[("g", t)] = (glo, ghi, ru4, ruf)

            def s1_tile(t):
                glo, ghi, ru4, ruf = state.pop(("g", t))
                hn = hi_n[t]
                # k-path fp8 channel-pair views + v-path f16 views
                glo8 = glo[0:64, 0, :].bitcast(FP8)   # [64, 2*NPAIR]
                glo16 = glo[:, 1, :]                  # [128, NPAIR] f16
                ghi8 = ghi[0:64, 0, :].bitcast(FP8) if hn else None
                ghi16 = ghi[:, 1, :] if hn else None

                NCH = NPAIR // CHUNK
                vpss, wpss, rts = [], [], []

                # ---- phase A: all front matmuls (v path + w accumulation),
                # with the r relu queued on ACT right after each chunk's wps
                for ch in range(NCH):
                    c0 = ch * CHUNK
                    vps = psV.tile([C, CHUNK], F32, tag="vps")
                    vpss.append(vps)
                    for hf in range(CHUNK // MM):
                        lo = c0 + hf * MM
                        o = vps[:, hf * MM:(hf + 1) * MM]
                        nc.tensor.matmul(o, wv[:], glo16[:, lo:lo + MM],
                                         start=True, stop=False)
                        nc.tensor.matmul(
                            o, wp2v[:], ruf[:, lo:lo + MM],
                            start=False, stop=(hn == 0 or lo + MM <= NPAIR - hn))
                        if hn and lo + MM > NPAIR - hn:
                            ho = max(NPAIR - hn, lo)
                            hb = NPAIR - hn
                            nc.tensor.matmul(
                                vps[:, ho - c0:(hf + 1) * MM], wv[:],
                                ghi16[:, ho - hb:lo + MM - hb],
                                start=False, stop=True)
                    wps = psW.tile([C, CHUNK], F32, tag="wps")
                    wpss.append(wps)
                    for hf in range(CHUNK // MM):
                        lo = c0 + hf * MM
                        o = wps[:, hf * MM:(hf + 1) * MM]
                        nc.tensor.matmul(
                            o, wk[:],
                            glo8[:, 2 * lo:2 * (lo + MM)]
                            .rearrange("p (n two) -> p two n", two=2),
                            start=True, stop=False, perf_mode=DR)
                        qv = (ftw[:, t * PT_TILE:(t + 1) * PT_TILE]
                              .unsqueeze(1).broadcast_to([C, MM // PT_TILE,
                                                          PT_TILE]))
                        nc.tensor.matmul(o, wq[:], qv, start=False, stop=False)
                        nc.tensor.matmul(
                            o, wp2w[:],
                            ru4[:, 2 * lo:2 * (lo + MM)]
                            .rearrange("p (n two) -> p two n", two=2),
                            start=False, stop=(hn == 0 or lo + MM <= NPAIR - hn),
                            perf_mode=DR)
                        if hn and lo + MM > NPAIR - hn:
                            ho = max(NPAIR - hn, lo)
                            hb = NPAIR - hn
                            nc.tensor.matmul(
                                wps[:, ho - c0:(hf + 1) * MM], wk[:],
                                ghi8[:, 2 * (ho - hb):2 * (lo + MM - hb)]
                                .rearrange("p (n two) -> p two n", two=2),
                                start=False, stop=True, perf_mode=DR)
                    r_t = wpool.tile([C, CHUNK], F16, tag=f"r{ch}")
                    rts.append(r_t)
                    nc.scalar.activation(r_t[:], wps[:], Relu,
                                         bias=b_w[:], scale=1.0 / SCALE)

                # ---- phase B: h ladder + exp per chunk
                ets = []
                for ch in range(NCH):
                    r_t = rts[ch]
                    hps = psX.tile([C, 2 * PT_TILE], F32, tag="scr")
                    for g in range(4):
                        nc.tensor.matmul(
                            hps[32 * g:32 * g + 32, :], ww1z[:],
                            r_t[:, 2 * g * PT_TILE:2 * (g + 1) * PT_TILE],
                            start=True, stop=True, tile_position=(0, 32 * g))
                    h2 = wpool.tile([C, 2 * PT_TILE], F16, tag="h2")
                    nc.scalar.activation(h2[:], hps[:], Relu, bias=b1f[:])
                    lps = psX.tile([C, CHUNK], F32, tag="scr")
                    for g in range(4):
                        nc.tensor.matmul(
                            lps[:, 2 * g * PT_TILE:2 * (g + 1) * PT_TILE],
                            w2r[:, g * C:(g + 1) * C], h2[:],
                            start=True, stop=True)
                    et = wpool.tile([C, 2, CHUNK], F16, tag=f"et{ch}")
                    ets.append(et)
                    nc.scalar.activation(et[:, 0, :], lps[:], Exp, bias=be_b[:])

                # ---- phase C: t2 + fused trees per chunk
                SP = []
                for ch in range(NCH):
                    et = ets[ch]
                    nc.vector.tensor_tensor(et[:, 1, :], et[:, 0, :],
                                            vpss[ch][:], MULT)
                    ev = et[:].rearrange("p u (a b) -> p u a b", b=PT_TILE)
                    f4 = wpool.tile([C, 2, 4 * PT_TILE], F16, tag="f4")
                    f4v = f4[:].rearrange("p u (a b) -> p u a b", b=PT_TILE)
                    nc.vector.tensor_tensor(f4v, ev[:, :, 0:4], ev[:, :, 4:8],
                                            ADD)
                    f8 = wpool.tile([C, 2, 2 * PT_TILE], F16, tag="f8")
                    f8v = f8[:].rearrange("p u (a b) -> p u a b", b=PT_TILE)
                    nc.vector.tensor_tensor(f8v, f4v[:, :, 0:2], f4v[:, :, 2:4],
                                            ADD)
                    sa = tpool.tile([C, 2, PT_TILE], F16, tag=f"SA{ch}")
                    nc.vector.tensor_tensor(sa[:], f8v[:, :, 0], f8v[:, :, 1],
                                            ADD)
                    SP.append(sa)

                # ---- tail
                su = tpool.tile([C, 2, PT_TILE], F16, tag="su")
                nc.vector.tensor_tensor(su[:], SP[0][:], SP[1][:], ADD)
                rS = tpool.tile([C, PT_TILE], F16, tag="rS")
                with nc.allow_low_precision("softmax denom recip in f16"):
                    nc.vector.reciprocal(rS[:], su[:, 0, :])
                # aggN = aggU * rS / SCALE  (rS is 1/(S_true/SCALE))
                aggN = tpool.tile([C, PT_TILE], F16, tag="aggN")
                nc.vector.scalar_tensor_tensor(aggN[:], su[:, 1, :],
                                               1.0 / SCALE, rS[:], MULT, MULT)
                l2 = tpool.tile([C, PT_TILE], F16, tag="l2")
                nc.vector.tensor_tensor(l2[:], aggN[:],
                                        ftw[:, bass.ts(t, PT_TILE)], ADD)
                outc = tpool.tile([C, PT_TILE], F32, tag="outc")
                nc.vector.scalar_tensor_tensor(outc[:], l2[:], 0.1, l2[:],
                                               MULT, MAX)
                nc.sync.dma_start(outT[:, bass.ts(t, PT_TILE)], outc[:])

            for i in range(n_tiles + 2):
                if i < n_tiles:
                    s0_gather(i)
                if i >= 2:
                    s1_tile(i - 2)

    from concourse.library_overlay import lower_extended_insts
    lower_extended_insts(nc)
    split_excess_waits(nc)
    return nc


# ------------------------------------------------------------- entry point
_CACHE = {}


def kernel(**inputs) -> np.ndarray:
    from concourse.bass_utils import run_bass_kernel_spmd

    xyz = np.asarray(inputs["xyz"], np.float32)
    feats = np.asarray(inputs["feats"], np.float32)
    nei = np.asarray(inputs["nei_ind"])
    params = {k: np.asarray(v, np.float32) for k, v in inputs.items()
              if k not in ("xyz", "feats", "nei_ind")}

    in_maps, meta = prep_inputs(xyz, feats, nei, params, N_CORES)

    # one compiled program per distinct hi_n profile; all cores share the
    # max profile so a single SPMD binary serves all 8
    hi_max = tuple(int(max(meta["hi_ns"][c][t] for c in range(N_CORES)))
                   for t in range(meta["n_tiles"]))
    key = (meta["n_tiles"], meta["per_core"], hi_max)
    if key not in _CACHE:
        _CACHE[key] = build_nc(meta, hi_max)
    nc = _CACHE[key]

    # build each core's hi idx stream against the shared profile
    hi_off_shared = np.concatenate([[0], np.cumsum(hi_max)]).astype(int)
    tot = max(int(hi_off_shared[-1]) // 16, 8)
    for cidx in range(N_CORES):
        dst = np.zeros((128, tot), np.int16)
        ent_core = meta["ents"][cidx]
        for t in range(meta["n_tiles"]):
            hn = hi_max[t]
            if hn:
                hv = ent_core[t][NPAIR - hn:]
                hv = np.where(hv >= THRESH, hv - THRESH + 1, 0
                              ).astype(np.int16)
                dst[:, hi_off_shared[t] // 16:hi_off_shared[t + 1] // 16] = \
                    meta["wrap"](hv)
        in_maps[cidx]["idx_hi"] = dst

    res = run_bass_kernel_spmd(nc, in_maps, core_ids=list(range(N_CORES)))
    outs = []
    for cidx, r in enumerate(res.results):
        o = np.asarray(r["outT"]).T                      # [per_core, C]
        inv = np.empty_like(meta["perms"][cidx])
        inv[meta["perms"][cidx]] = np.arange(len(inv))
        outs.append(o[inv])
    full = np.concatenate(outs, axis=0)                  # [npad, C]
    return np.ascontiguousarray(full[None, :meta["n_real"]]).astype(np.float32)


# revision 42
# speedup vs baseline: 1.4479x; 1.0026x over previous
"""PointTransformerLayer Bass kernel for TRN2 (v2).

Per-core design (points sharded 8 ways, table replicated):
  - DRAM gather table: one 256B row per point = 128 fp8(e4m3) feats packed
    as channel pairs (u16 word w = channels 2w, 2w+1).  Gathered in
    transpose mode the tile is natively in fp8-DoubleRow moving-operand
    layout, so Wk/Wv matmuls run at 0.5 cyc/col.
  - int16 index reach is 32767 rows, so the table is split lo/hi with a
    zero row at index 0.  Per point the 16 neighbor slots are reordered
    (softmax over K is permutation invariant) so hi-table refs occupy the
    top k-blocks; points are sorted per core by hi-count so each 128-pt
    tile needs only hi_n[t] = 128*max_hi descriptors for the hi gather.
    No on-chip merge: lo and hi gathered tiles are separately accumulated
    into PSUM by the (linear) Wk/Wv matmuls; missing slots fetch zero rows.
  - Pair order within a tile is k-major: col = 128*k + pt.
  - The position term relu(a_nbr - a_ctr) is precomputed on host and
    streamed as a [2,2,N] fp8 DoubleRow operand (3 channels + ones; the
    ones row carries nothing for w (bias via ACT) and 16*bvp for v).
  - All fp8 weights carry a x16 scale; undone via ACT scale (w path),
    exp bias (e path) and reciprocal scale (normalization).
  - Back-end packing: h = relu(Ww1'r) lands as [128,128] PSUM (8 matmuls
    with partition offsets), Ww2 is applied block-diagonally (128 cols),
    exp runs on [128,128], softmax denom comes from a one-hot reduction
    matmul accumulated across chunks, and e is re-broadcast to [128,1024]
    PSUM by 8 one-hot matmuls.
  - t2 = e*v on DVE straight from PSUM; K-reduction = 3-level strided
    tree adds in f16.
"""

import sys

sys.path.insert(0, "/opt/trn_rl_repo")
sys.path.insert(0, "/root/.axon_site/_ro/trn_rl_repo")

import numpy as np
import ml_dtypes

import concourse.bass as bass
import concourse.tile as tile
from concourse import mybir

F16 = mybir.dt.float16
F32 = mybir.dt.float32
FP8 = mybir.dt.float8e4
I16 = mybir.dt.int16
NPF8 = ml_dtypes.float8_e4m3

K = 16
C = 128
S = 8
CS = C // S  # 16
EPS = 1e-5
EXP_SHIFT = float(np.log(256.0))
SCALE = 16.0
LN_SCALE = float(np.log(SCALE))
PT_TILE = 128
NPAIR = PT_TILE * K     # 2048
CHUNK = 1024
MM = 512
NBLK = NPAIR // PT_TILE  # 16 k-blocks per tile
THRESH = 32767           # entries in lo table (idx = e+1 <= 32767)

N_CORES = 8


# ----------------------------------------------------------------- host math
def fold_params(p):
    f32 = np.float32
    s_p = (p["p_gamma"] / np.sqrt(p["p_var"] + EPS)).astype(f32)
    Afold = (p["Wp1"] * s_p[None, :]).astype(f32)
    cfold = ((p["bp1"] - p["p_mean"]) * s_p + p["p_beta"]).astype(f32)

    s_w = (p["w_gamma"] / np.sqrt(p["w_var"] + EPS)).astype(f32)
    ball = (p["bk"] - p["bq"] + p["bp2"]).astype(f32)
    b_w = ((ball - p["w_mean"]) * s_w + p["w_beta"]).astype(f32)

    s1 = (p["w1_gamma"] / np.sqrt(p["w1_var"] + EPS)).astype(f32)
    ww1s = (p["Ww1"] * s1[None, :]).astype(np.float16)        # [128, 16]
    b1f = ((p["bw1"] - p["w1_mean"]) * s1 + p["w1_beta"]).astype(f32)

    bvp = (p["bv"] + p["bp2"]).astype(f32)                    # [128]

    def pack_dr(w):  # [in_ch, out] f32 -> [in_ch//2, 2, out] fp8
        ic = w.shape[0]
        return np.ascontiguousarray(
            w.reshape(ic // 2, 2, w.shape[1])).astype(NPF8)

    wk_dr = pack_dr(SCALE * s_w[None, :] * p["Wk"])           # [64,2,128]
    wv_f16 = (SCALE * p["Wv"]).astype(np.float16)             # [128,128]
    wq_f16 = (-SCALE * s_w[None, :] * p["Wq"]).astype(np.float16)  # [128,128]

    # ru4 channels: 0..2 = relu(u), 3 = ones
    wp2w = np.zeros((4, C), np.float32)
    wp2w[:3] = SCALE * s_w[None, :] * p["Wp2"]
    wp2w_dr = pack_dr(wp2w)                                   # [2,2,128]
    wp2v = np.zeros((4, C), np.float32)
    wp2v[:3] = SCALE * p["Wp2"]
    wp2v[3] = SCALE * bvp
    wp2v_f16 = wp2v.astype(np.float16)                        # [4,128]

    # packed h layout: 4 groups of 32 partitions (16 real + 16 zero hole);
    # group g covers pair-columns [256g, 256g+256) of a 1024-col chunk.
    ww1z = np.zeros((C, 32), np.float16)       # Ww1 + 16 zero out-cols
    ww1z[:, :16] = ww1s
    b1f_pk = np.zeros(C, f32)
    for g in range(4):
        b1f_pk[32 * g:32 * g + 16] = b1f
    # replicated-logits stationaries: block g maps packed h (partitions
    # 32g+i) to all 128 replicated logit channels for its 256 columns
    w2r = np.zeros((C, 4 * C), np.float16)
    for g in range(4):
        for c in range(C):
            for i in range(16):
                w2r[32 * g + i, g * C + c] = p["Ww2"][i, c % 16]
    be_rep = (np.array([p["bw2"][c % 16] for c in range(C)], f32)
              - EXP_SHIFT - LN_SCALE)

    return dict(
        Afold=Afold, cfold=cfold, b_w=b_w, ww1z=ww1z,
        wk_dr=wk_dr, wv_f16=wv_f16, wq_f16=wq_f16,
        wp2w_dr=wp2w_dr, wp2v_f16=wp2v_f16,
        be_rep=be_rep, b1f_pk=b1f_pk, w2r=w2r,
    )


def prep_inputs(xyz, feats, nei_ind, params, n_cores):
    f = fold_params(params)
    n_real = feats.shape[1]
    per_core_raw = -(-n_real // n_cores)
    per_core = -(-per_core_raw // PT_TILE) * PT_TILE
    npad = per_core * n_cores
    n_tiles = per_core // PT_TILE

    feats0 = np.zeros((npad, C), np.float32)
    feats0[:n_real] = feats[0]
    pos0 = np.zeros((npad, 3), np.float32)
    pos0[:n_real] = xyz[0]
    ni = np.zeros((npad, K), np.int64)
    ni[:n_real] = nei_ind[0]

    a = (pos0 @ f["Afold"]).astype(np.float32)            # [npad, 3]
    actr = (a - f["cfold"][None, :]).astype(np.float32)   # center role

    # table rows (512B): words 0..63 = fp8 channel-paired feats (k path),
    # words 128..255 = f16 feats (v path)
    feats8 = feats0.astype(NPF8)                          # [npad, 128] fp8
    ent = np.zeros((npad, 256), np.uint16)
    ent[:, :64] = feats8.view(np.uint8)[:, 0::2].astype(np.uint16) \
        | (feats8.view(np.uint8)[:, 1::2].astype(np.uint16) << 8)
    ent[:, 128:256] = feats0.astype(np.float16).view(np.uint16)
    lo_rows = THRESH + 1
    hi_rows = npad - THRESH + 1
    table_lo = np.zeros((lo_rows, 256), np.uint16)
    table_lo[1:] = ent[:THRESH]
    table_hi = np.zeros((hi_rows, 256), np.uint16)
    table_hi[1:] = ent[THRESH:]
    table_lo = table_lo.view(np.float16)
    table_hi = table_hi.view(np.float16)

    # per-point slot sort: lo refs first, hi refs last
    is_hi = ni >= THRESH                                  # [npad, K]
    slot_order = np.argsort(is_hi, axis=1, kind="stable") # [npad, K]
    ni_sorted = np.take_along_axis(ni, slot_order, axis=1)
    hi_cnt = is_hi.sum(axis=1)                            # [npad]

    def wrap(arr):
        # arr [n] -> [128, n//16] int16 (16-wrap replicated to 128)
        n = arr.shape[0]
        w16 = arr.reshape(n // 16, 16).T.astype(np.int16)
        return np.tile(w16, (8, 1))

    in_maps = []
    metas = []
    for cidx in range(n_cores):
        sl = slice(cidx * per_core, (cidx + 1) * per_core)
        # sort this core's points by hi-count so tiles are homogeneous
        hc = hi_cnt[sl]
        perm = np.argsort(hc, kind="stable")              # local indices
        gperm = cidx * per_core + perm                    # global ids
        nis = ni_sorted[gperm]                            # [per_core, K]

        hi_n = []
        lo_cols = np.zeros((128, n_tiles * 128), np.int16)
        ent_core = np.zeros((n_tiles, NPAIR), np.int64)
        ru_blocks = np.zeros((2, n_tiles * 2 * NPAIR), NPF8)
        ruf_blocks = np.zeros((4, n_tiles * NPAIR), np.float16)
        for t in range(n_tiles):
            pts = nis[t * PT_TILE:(t + 1) * PT_TILE]      # [128, K]
            # k-major pair order: col j = 128*k + pt -> entry pts[pt, k]
            ent_t = pts.T.reshape(-1)                     # [2048] (k-major)
            ent_core[t] = ent_t
            lo_v = np.where(ent_t < THRESH, ent_t + 1, 0).astype(np.int16)
            lo_cols[:, t * 128:(t + 1) * 128] = wrap(lo_v)
            mh = int(hi_cnt[gperm[t * PT_TILE:(t + 1) * PT_TILE]].max())
            hi_n.append(128 * mh)
            # ru for this tile, k-major
            ctr = np.repeat(gperm[t * PT_TILE:(t + 1) * PT_TILE][None, :],
                            K, axis=0).reshape(-1)        # [2048]
            u = a[ent_t] - actr[ctr]                      # [2048, 3]
            ru = np.maximum(u, 0.0).astype(np.float32)
            ru4 = np.concatenate([ru, np.ones((NPAIR, 1), np.float32)],
                                 axis=1)                  # [2048, 4]
            blk = ru4.reshape(NPAIR, 2, 2).transpose(1, 0, 2)  # [2, 2048, 2]
            ru_blocks[:, t * 2 * NPAIR:(t + 1) * 2 * NPAIR] = \
                blk.reshape(2, 2 * NPAIR).astype(NPF8)
            ruf_blocks[:, t * NPAIR:(t + 1) * NPAIR] = ru4.T.astype(np.float16)

        ftw = np.ascontiguousarray(feats0[gperm].T.astype(np.float16))

        in_maps.append({
            "table_lo": table_lo, "table_hi": table_hi,
            "idx_lo": lo_cols,
            "ru4": ru_blocks, "ruf": ruf_blocks, "ftw": ftw,
            "wk_dr": f["wk_dr"], "wv_f16": f["wv_f16"],
            "wq_f16": f["wq_f16"],
            "wp2w_dr": f["wp2w_dr"], "wp2v_f16": f["wp2v_f16"],
            "ww1z": f["ww1z"], "w2r": f["w2r"],
            "b_w": f["b_w"].reshape(C, 1),
            "b1f_pk": f["b1f_pk"].reshape(C, 1),
            "be_rep": f["be_rep"].reshape(C, 1),
        })
        metas.append(dict(hi_n=tuple(hi_n), perm=perm, ent=ent_core,
                          wrap=wrap))

    meta = dict(n_tiles=n_tiles, per_core=per_core, npad=npad,
                lo_rows=lo_rows, hi_rows=hi_rows, n_real=n_real,
                hi_ns=tuple(m["hi_n"] for m in metas),
                perms=[m["perm"] for m in metas],
                ents=[m["ent"] for m in metas], wrap=wrap)
    return in_maps, meta


# ------------------------------------------------------------- walrus compat
def split_excess_waits(nc, max_waits=1):
    n_split = 0
    for fn in nc.m.functions:
        for blk in fn.blocks:
            new_insts = []
            for inst in blk.instructions:
                si = inst.sync_info
                lim = (1 if isinstance(inst, (mybir.InstDrain, mybir.InstNoOp,
                                              mybir.InstEventSemaphore))
                       else max_waits)
                if si is not None and si.on_wait and len(si.on_wait) > lim:
                    waits = list(si.on_wait)
                    extra, keep = waits[:-lim], waits[-lim:]
                    ci = 0
                    while extra:
                        chunk, extra = extra[:1], extra[1:]
                        new_insts.append(mybir.InstNoOp(
                            name=f"{inst.name}-waitsplit{ci}",
                            engine=inst.engine,
                            bass_nofuse=True,
                            sync_info=mybir.SyncInfo(on_wait=chunk, on_update=[]),
                        ))
                        ci += 1
                    si.on_wait = keep
                    n_split += 1
                new_insts.append(inst)
            blk.instructions = new_insts
    return n_split


# ----------------------------------------------------------------- the kernel
def build_nc(meta, hi_n):
    from concourse import library_config
    n_tiles = meta["n_tiles"]
    per_core = meta["per_core"]
    hi_tot = sum(hi_n)
    hi_off = np.concatenate([[0], np.cumsum(hi_n)]).astype(int)
    nc = bass.Bass("TRN2", target_bir_lowering=False, debug=False,
                   num_swdge_queues=1)

    dt_ = nc.dram_tensor
    t_lo = dt_("table_lo", [meta["lo_rows"], 256], F16, kind="ExternalInput").ap()
    t_hi = dt_("table_hi", [meta["hi_rows"], 256], F16, kind="ExternalInput").ap()
    idx_lo = dt_("idx_lo", [128, n_tiles * 128], I16, kind="ExternalInput").ap()
    idx_hi_cols = max(hi_tot // 16, 8)
    idx_hi = dt_("idx_hi", [128, idx_hi_cols], I16, kind="ExternalInput").ap()
    ru4_d = dt_("ru4", [2, n_tiles * 2 * NPAIR], FP8, kind="ExternalInput").ap()
    ruf_d = dt_("ruf", [4, n_tiles * NPAIR], F16, kind="ExternalInput").ap()
    ftw_d = dt_("ftw", [C, per_core], F16, kind="ExternalInput").ap()
    wk_d = dt_("wk_dr", [64, 2, C], FP8, kind="ExternalInput").ap()
    wv_d = dt_("wv_f16", [C, C], F16, kind="ExternalInput").ap()
    wq_d = dt_("wq_f16", [C, C], F16, kind="ExternalInput").ap()
    wp2w_d = dt_("wp2w_dr", [2, 2, C], FP8, kind="ExternalInput").ap()
    wp2v_d = dt_("wp2v_f16", [4, C], F16, kind="ExternalInput").ap()
    ww1z_d = dt_("ww1z", [C, 32], F16, kind="ExternalInput").ap()
    w2r_d = dt_("w2r", [C, 4 * C], F16, kind="ExternalInput").ap()
    b_w_d = dt_("b_w", [C, 1], F32, kind="ExternalInput").ap()
    b1f_d = dt_("b1f_pk", [C, 1], F32, kind="ExternalInput").ap()
    be_d = dt_("be_rep", [C, 1], F32, kind="ExternalInput").ap()
    outT = dt_("outT", [C, per_core], F32, kind="ExternalOutput").ap()

    Relu = mybir.ActivationFunctionType.Relu
    Exp = mybir.ActivationFunctionType.Exp
    DR = mybir.MatmulPerfMode.DoubleRow
    ADD = mybir.AluOpType.add
    MULT = mybir.AluOpType.mult
    MAX = mybir.AluOpType.max

    nc.gpsimd.load_library(library_config.mlp)
    rlo = nc.gpsimd.alloc_register("nlo")
    nc.gpsimd.reg_mov(rlo, NPAIR)
    rhi = nc.gpsimd.alloc_register("nhi")

    with tile.TileContext(nc) as tc:
        with (
            tc.tile_pool(name="const", bufs=1) as cpool,
            tc.tile_pool(name="gath", bufs=3) as gpool,
            tc.tile_pool(name="ru", bufs=3) as rupool,
            tc.tile_pool(name="work", bufs=2) as wpool,
            tc.tile_pool(name="tail", bufs=2) as tpool,
            tc.tile_pool(name="psW", bufs=1, space="PSUM") as psW,
            tc.tile_pool(name="psV", bufs=2, space="PSUM") as psV,
            tc.tile_pool(name="psX", bufs=1, space="PSUM") as psX,
        ):
            def cload(ap_dram, shape, dtype, tag):
                t = cpool.tile(shape, dtype, tag=tag)
                nc.sync.dma_start(t[:], ap_dram)
                return t

            wk = cload(wk_d, [64, 2, C], FP8, "wk")
            wv = cload(wv_d, [C, C], F16, "wv")
            wq = cload(wq_d, [C, C], F16, "wq")
            wp2w = cload(wp2w_d, [2, 2, C], FP8, "wp2w")
            wp2v = cload(wp2v_d, [4, C], F16, "wp2v")
            ww1z = cload(ww1z_d, [C, 32], F16, "ww1z")
            w2r = cload(w2r_d, [C, 4 * C], F16, "w2r")
            b_w = cload(b_w_d, [C, 1], F32, "b_w")
            b1f = cload(b1f_d, [C, 1], F32, "b1f")
            be_b = cload(be_d, [C, 1], F32, "be_b")
            ixlo = cload(idx_lo, [128, n_tiles * 128], I16, "ixlo")
            ixhi = cload(idx_hi, [128, idx_hi_cols], I16, "ixhi")
            ftw = cpool.tile([C, per_core], F16, tag="ftw")
            nc.sync.dma_start(ftw[:], ftw_d)

            state = {}

            # PE p-state warmup: ~6.5us of back-to-back dummy matmuls ramps
            # the tensor engine to full clock (persists across <3us gaps)
            # and covers the first tile's gather latency.
            wu_ps = psX.tile([C, MM], F32, tag="scr")
            for _ in range(24):
                nc.tensor.matmul(wu_ps[:], wq[:], w2r[:, 0:MM],
                                 start=True, stop=True)

            def s0_gather(t):
                glo = gpool.tile([128, 2, NPAIR], F16, tag="glo")
                nc.gpsimd.dma_gather(glo[:], t_lo,
                                     ixlo[:, bass.ts(t, 128)], NPAIR, rlo,
                                     256, transpose=True, queue_num=0,
                                     single_packet=False)
                hn = hi_n[t]
                ghi = None
                if hn > 0:
                    ghi = gpool.tile([128, 2, hn], F16, tag="ghi")
                    nc.gpsimd.reg_mov(rhi, hn)
                    nc.gpsimd.dma_gather(
                        ghi[:], t_hi,
                        ixhi[:, hi_off[t] // 16:hi_off[t + 1] // 16], hn, rhi,
                        256, transpose=True, queue_num=0,
                        single_packet=False)
                ru4 = rupool.tile([2, 2 * NPAIR], FP8, tag="ru4")
                nc.sync.dma_start(ru4[:],
                                  ru4_d[:, t * 2 * NPAIR:(t + 1) * 2 * NPAIR])
                ruf = rupool.tile([4, NPAIR], F16, tag="ruf")
                nc.sync.dma_start(ruf[:], ruf_d[:, t * NPAIR:(t + 1) * NPAIR])
                state[("g", t)] = (glo, ghi, ru4, ruf)

            def s1_tile(t):
                glo, ghi, ru4, ruf = state.pop(("g", t))
                hn = hi_n[t]
                # k-path fp8 channel-pair views + v-path f16 views
                glo8 = glo[0:64, 0, :].bitcast(FP8)   # [64, 2*NPAIR]
                glo16 = glo[:, 1, :]                  # [128, NPAIR] f16
                ghi8 = ghi[0:64, 0, :].bitcast(FP8) if hn else None
                ghi16 = ghi[:, 1, :] if hn else None

                NCH = NPAIR // CHUNK
                vpss, wpss, rts = [], [], []

                # ---- phase A: all front matmuls (v path + w accumulation),
                # with the r relu queued on ACT right after each chunk's wps
                for ch in range(NCH):
                    c0 = ch * CHUNK
                    vps = psV.tile([C, CHUNK], F32, tag="vps")
                    vpss.append(vps)
                    for hf in range(CHUNK // MM):
                        lo = c0 + hf * MM
                        o = vps[:, hf * MM:(hf + 1) * MM]
                        nc.tensor.matmul(o, wv[:], glo16[:, lo:lo + MM],
                                         start=True, stop=False)
                        nc.tensor.matmul(
                            o, wp2v[:], ruf[:, lo:lo + MM],
                            start=False, stop=(hn == 0 or lo + MM <= NPAIR - hn))
                        if hn and lo + MM > NPAIR - hn:
                            ho = max(NPAIR - hn, lo)
                            hb = NPAIR - hn
                            nc.tensor.matmul(
                                vps[:, ho - c0:(hf + 1) * MM], wv[:],
                                ghi16[:, ho - hb:lo + MM - hb],
                                start=False, stop=True)
                    wps = psW.tile([C, CHUNK], F32, tag="wps")
                    wpss.append(wps)
                    for hf in range(CHUNK // MM):
                        lo = c0 + hf * MM
                        o = wps[:, hf * MM:(hf + 1) * MM]
                        nc.tensor.matmul(
                            o, wk[:],
                            glo8[:, 2 * lo:2 * (lo + MM)]
                            .rearrange("p (n two) -> p two n", two=2),
                            start=True, stop=False, perf_mode=DR)
                        qv = (ftw[:, t * PT_TILE:(t + 1) * PT_TILE]
                              .unsqueeze(1).broadcast_to([C, MM // PT_TILE,
                                                          PT_TILE]))
                        nc.tensor.matmul(o, wq[:], qv, start=False, stop=False)
                        nc.tensor.matmul(
                            o, wp2w[:],
                            ru4[:, 2 * lo:2 * (lo + MM)]
                            .rearrange("p (n two) -> p two n", two=2),
                            start=False, stop=(hn == 0 or lo + MM <= NPAIR - hn),
                            perf_mode=DR)
                        if hn and lo + MM > NPAIR - hn:
                            ho = max(NPAIR - hn, lo)
                            hb = NPAIR - hn
                            nc.tensor.matmul(
                                wps[:, ho - c0:(hf + 1) * MM], wk[:],
                                ghi8[:, 2 * (ho - hb):2 * (lo + MM - hb)]
                                .rearrange("p (n two) -> p two n", two=2),
                                start=False, stop=True, perf_mode=DR)
                    r_t = wpool.tile([C, CHUNK], F16, tag=f"r{ch}")
                    rts.append(r_t)
                    nc.scalar.activation(r_t[:], wps[:], Relu,
                                         bias=b_w[:], scale=1.0 / SCALE)

                # ---- phase B: h ladder + exp per chunk
                ets = []
                for ch in range(NCH):
                    r_t = rts[ch]
                    hps = psX.tile([C, 2 * PT_TILE], F32, tag="scr")
                    for g in range(4):
                        nc.tensor.matmul(
                            hps[32 * g:32 * g + 32, :], ww1z[:],
                            r_t[:, 2 * g * PT_TILE:2 * (g + 1) * PT_TILE],
                            start=True, stop=True, tile_position=(0, 32 * g))
                    h2 = wpool.tile([C, 2 * PT_TILE], F16, tag="h2")
                    nc.scalar.activation(h2[:], hps[:], Relu, bias=b1f[:])
                    lps = psX.tile([C, CHUNK], F32, tag="scr")
                    for g in range(4):
                        nc.tensor.matmul(
                            lps[:, 2 * g * PT_TILE:2 * (g + 1) * PT_TILE],
                            w2r[:, g * C:(g + 1) * C], h2[:],
                            start=True, stop=True)
                    et = wpool.tile([C, 2, CHUNK], F16, tag=f"et{ch}")
                    ets.append(et)
                    nc.scalar.activation(et[:, 0, :], lps[:], Exp, bias=be_b[:])

                # ---- phase C: t2 + fused trees per chunk
                SP = []
                for ch in range(NCH):
                    et = ets[ch]
                    nc.vector.tensor_tensor(et[:, 1, :], et[:, 0, :],
                                            vpss[ch][:], MULT)
                    ev = et[:].rearrange("p u (a b) -> p u a b", b=PT_TILE)
                    f4 = wpool.tile([C, 2, 4 * PT_TILE], F16, tag="f4")
                    f4v = f4[:].rearrange("p u (a b) -> p u a b", b=PT_TILE)
                    nc.vector.tensor_tensor(f4v, ev[:, :, 0:4], ev[:, :, 4:8],
                                            ADD)
                    f8 = wpool.tile([C, 2, 2 * PT_TILE], F16, tag="f8")
                    f8v = f8[:].rearrange("p u (a b) -> p u a b", b=PT_TILE)
                    nc.vector.tensor_tensor(f8v, f4v[:, :, 0:2], f4v[:, :, 2:4],
                                            ADD)
                    sa = tpool.tile([C, 2, PT_TILE], F16, tag=f"SA{ch}")
                    nc.vector.tensor_tensor(sa[:], f8v[:, :, 0], f8v[:, :, 1],
                                            ADD)
                    SP.append(sa)

                # ---- tail
                su = tpool.tile([C, 2, PT_TILE], F16, tag="su")
                nc.vector.tensor_tensor(su[:], SP[0][:], SP[1][:], ADD)
                rS = tpool.tile([C, PT_TILE], F16, tag="rS")
                with nc.allow_low_precision("softmax denom recip in f16"):
                    nc.vector.reciprocal(rS[:], su[:, 0, :])
                # aggN = aggU * rS / SCALE  (rS is 1/(S_true/SCALE))
                aggN = tpool.tile([C, PT_TILE], F16, tag="aggN")
                nc.vector.scalar_tensor_tensor(aggN[:], su[:, 1, :],
                                               1.0 / SCALE, rS[:], MULT, MULT)
                l2 = tpool.tile([C, PT_TILE], F16, tag="l2")
                nc.vector.tensor_tensor(l2[:], aggN[:],
                                        ftw[:, bass.ts(t, PT_TILE)], ADD)
                outc = tpool.tile([C, PT_TILE], F32, tag="outc")
                nc.vector.scalar_tensor_tensor(outc[:], l2[:], 0.1, l2[:],
                                               MULT, MAX)
                nc.sync.dma_start(outT[:, bass.ts(t, PT_TILE)], outc[:])

            for i in range(n_tiles + 2):
                if i < n_tiles:
                    s0_gather(i)
                if i >= 2:
                    s1_tile(i - 2)

    from concourse.library_overlay import lower_extended_insts
    lower_extended_insts(nc)
    split_excess_waits(nc)
    return nc


# ------------------------------------------------------------- entry point
_CACHE = {}


def kernel(**inputs) -> np.ndarray:
    from concourse.bass_utils import run_bass_kernel_spmd

    xyz = np.asarray(inputs["xyz"], np.float32)
    feats = np.asarray(inputs["feats"], np.float32)
    nei = np.asarray(inputs["nei_ind"])
    params = {k: np.asarray(v, np.float32) for k, v in inputs.items()
              if k not in ("xyz", "feats", "nei_ind")}

    in_maps, meta = prep_inputs(xyz, feats, nei, params, N_CORES)

    # one compiled program per distinct hi_n profile; all cores share the
    # max profile so a single SPMD binary serves all 8
    hi_max = tuple(int(max(meta["hi_ns"][c][t] for c in range(N_CORES)))
                   for t in range(meta["n_tiles"]))
    key = (meta["n_tiles"], meta["per_core"], hi_max)
    if key not in _CACHE:
        _CACHE[key] = build_nc(meta, hi_max)
    nc = _CACHE[key]

    # build each core's hi idx stream against the shared profile
    hi_off_shared = np.concatenate([[0], np.cumsum(hi_max)]).astype(int)
    tot = max(int(hi_off_shared[-1]) // 16, 8)
    for cidx in range(N_CORES):
        dst = np.zeros((128, tot), np.int16)
        ent_core = meta["ents"][cidx]
        for t in range(meta["n_tiles"]):
            hn = hi_max[t]
            if hn:
                hv = ent_core[t][NPAIR - hn:]
                hv = np.where(hv >= THRESH, hv - THRESH + 1, 0
                              ).astype(np.int16)
                dst[:, hi_off_shared[t] // 16:hi_off_shared[t + 1] // 16] = \
                    meta["wrap"](hv)
        in_maps[cidx]["idx_hi"] = dst

    res = run_bass_kernel_spmd(nc, in_maps, core_ids=list(range(N_CORES)))
    outs = []
    for cidx, r in enumerate(res.results):
        o = np.asarray(r["outT"]).T                      # [per_core, C]
        inv = np.empty_like(meta["perms"][cidx])
        inv[meta["perms"][cidx]] = np.arange(len(inv))
        outs.append(o[inv])
    full = np.concatenate(outs, axis=0)                  # [npad, C]
    return np.ascontiguousarray(full[None, :meta["n_real"]]).astype(np.float32)
